# revision 1
# baseline (speedup 1.0000x reference)
"""Trainium2 Bass kernel for nn_AttentiveEncoderPOS (embed+concat+linear+self-attention).

Strategy (8 cores, SPMD, no collectives):
  - Each core receives input_ids/pos_ids ROTATED so that its 1024-row slice
    comes first. Softmax attention is invariant to key/value permutation, so
    each core computes the full L = concat(emb[ids], pos[pids]) @ W.T + b
    (keys/values, in its own order) and attends only its first 1024 rows
    (queries) against all 8192 keys. Output rows i*1024:(i+1)*1024 come from
    core i directly.
  - Layout: L is produced TRANSPOSED (L.T, h on partitions) by the linear
    matmul; scores are computed transposed (keys on partitions, q on free) so
    that exp(scores) feeds the A@V matmul directly as the stationary operand
    and the softmax denominator is a per-partition scale after A@V.
  - bf16 matmul inputs, fp32 PSUM accumulation. Scores are tiny (|s|<0.01)
    so exp() without max-subtraction is exact softmax.
"""

import numpy as np

import concourse.bass as bass
import concourse.mybir as mybir
from concourse import bacc
from concourse.tile import TileContext
from concourse.bass_utils import run_bass_kernel_spmd
from concourse.masks import make_identity

N = 8192
H = 1024
VOCAB = 50257
POS = 64
NCORES = 8
NL = N // NCORES          # 1024 query rows per core
P = 128
KT = N // P               # 64 key tiles
HT = H // P               # 8 h tiles
CHUNK = 512
NCH = N // CHUNK          # 16 phase-1 chunks
RT = CHUNK // P           # 4 row tiles / chunk
K2 = 2 * H
KTI = K2 // P             # 16 contraction tiles for the linear
QTN = NL // P             # 8 q tiles
BLK = 8                   # key tiles per phase-2 block (PSUM accum chain len)
NBLK = KT // BLK
SCALE = 1.0 / 32.0        # 1/sqrt(H)

BF = mybir.dt.bfloat16
F32 = mybir.dt.float32
I32 = mybir.dt.int32
EXP = mybir.ActivationFunctionType.Exp


def build_nc():
    nc = bacc.Bacc()
    ids = nc.declare_dram_parameter("ids", [KT, P, 1], I32, isOutput=False)
    pids = nc.declare_dram_parameter("pids", [KT, P, 1], I32, isOutput=False)
    emb = nc.declare_dram_parameter("emb", [VOCAB, H], F32, isOutput=False)
    pemb = nc.declare_dram_parameter("pemb", [POS, H], F32, isOutput=False)
    wt = nc.declare_dram_parameter("wt", [K2, H], F32, isOutput=False)  # W.T
    bias = nc.declare_dram_parameter("bias", [HT, P, 1], F32, isOutput=False)
    out = nc.declare_dram_parameter("out", [NL, H], F32, isOutput=True)

    # L.T tile-blocked: [key-tile][h-tile][128 h, 128 key] bf16
    lt_d = nc.dram_tensor("lt_d", [KT, HT, P, P], BF)
    # V (= L, natural layout): [key-tile][128 key, 1024 h] bf16
    v_d = nc.dram_tensor("v_d", [KT, P, H], BF)

    with TileContext(nc) as tc:
        with tc.tile_pool(name="const", bufs=1) as const:
            ident = const.tile([P, P], BF)
            make_identity(nc, ident[:])
            ones = const.tile([P, 1], BF)
            nc.gpsimd.memset(ones[:], 1.0)
            ident32 = const.tile([P, P], F32)
            make_identity(nc, ident32[:])
            b_row = const.tile([1, H], F32)
            nc.sync.dma_start(
                out=b_row[0:1, :], in_=bias.rearrange("h p u -> u (h p)")
            )
            b_sb = const.tile([P, HT], F32)
            nc.sync.dma_start(
                out=b_sb[:].rearrange("p (h u) -> p h u", h=HT),
                in_=bias.rearrange("h p u -> p h u"),
            )

            # ---------------- Phase 1: L.T and V production ----------------
            with (
                tc.tile_pool(name="wtp", bufs=KTI) as wtp,
                tc.tile_pool(name="wld", bufs=2) as wld,
                tc.tile_pool(name="idp", bufs=8) as idp,
                tc.tile_pool(name="xfp", bufs=3) as xfp,
                tc.tile_pool(name="xbp", bufs=RT + 2) as xbp,
                tc.tile_pool(name="xtp", bufs=2 * KTI) as xtp,
                tc.tile_pool(name="ltp", bufs=2 * HT) as ltp,
                tc.tile_pool(name="lup", bufs=2 * HT) as lup,
                tc.tile_pool(name="vp", bufs=8) as vp,
                tc.tile_pool(name="tps", bufs=3, space="PSUM") as tps,
                tc.tile_pool(name="mps", bufs=2, space="PSUM") as mps,
            ):
                # W.T -> bf16 SBUF, one [128, H] tile per contraction k-tile
                wtb = []
                for k in range(KTI):
                    wf = wld.tile([P, H], F32, tag="wld")
                    nc.sync.dma_start(out=wf[:], in_=wt[k * P : (k + 1) * P, :])
                    wb = wtp.tile([P, H], BF, tag="wtb")
                    nc.vector.tensor_copy(out=wb[:], in_=wf[:])
                    wtb.append(wb)

                for ch in range(NCH):
                    # gather + transpose X for this chunk of 512 rows
                    xts = []
                    for k in range(KTI):
                        xts.append(xtp.tile([P, CHUNK], BF, tag="xt", name="xt"))
                    xbs = []
                    for rt in range(RT):
                        t = ch * RT + rt
                        idt = idp.tile([P, 1], I32, tag="id")
                        nc.sync.dma_start(out=idt[:], in_=ids[t])
                        pidt = idp.tile([P, 1], I32, tag="pid")
                        nc.sync.dma_start(out=pidt[:], in_=pids[t])
                        xf = xfp.tile([P, K2], F32, tag="xf")
                        nc.gpsimd.indirect_dma_start(
                            out=xf[:, 0:H],
                            out_offset=None,
                            in_=emb[:],
                            in_offset=bass.IndirectOffsetOnAxis(ap=idt[:, :1], axis=0),
                        )
                        nc.gpsimd.indirect_dma_start(
                            out=xf[:, H:K2],
                            out_offset=None,
                            in_=pemb[:],
                            in_offset=bass.IndirectOffsetOnAxis(ap=pidt[:, :1], axis=0),
                        )
                        xb = xbp.tile([P, K2], BF, tag="xb")
                        nc.vector.tensor_copy(out=xb[:], in_=xf[:])
                        xbs.append(xb)
                    for k in range(KTI):
                        pt = tps.tile([P, CHUNK], BF, tag="tp")
                        for rt in range(RT):
                            nc.tensor.transpose(
                                pt[:, rt * P : (rt + 1) * P],
                                xbs[rt][:, k * P : (k + 1) * P],
                                ident[:],
                            )
                        nc.vector.tensor_copy(out=xts[k][:], in_=pt[:])

                    # linear: L.T[ht, chunk] = sum_k W.T[k,ht].T @ X.T[k,chunk]
                    lts = []
                    for ht in range(HT):
                        pm = mps.tile([P, CHUNK], F32, tag="mp")
                        for k in range(KTI):
                            nc.tensor.matmul(
                                pm[:],
                                lhsT=wtb[k][:, ht * P : (ht + 1) * P],
                                rhs=xts[k][:],
                                start=(k == 0),
                                stop=(k == KTI - 1),
                            )
                        lt = ltp.tile([P, CHUNK], BF, tag="lt")
                        nc.vector.tensor_add(
                            out=lt[:],
                            in0=pm[:],
                            in1=b_sb[:, ht : ht + 1].to_broadcast([P, CHUNK]),
                        )
                        ltu = lup.tile([P, CHUNK], BF, tag="ltu")
                        nc.vector.tensor_copy(out=ltu[:], in_=pm[:])
                        lts.append(ltu)
                        nc.sync.dma_start(
                            out=lt_d[ch * RT : (ch + 1) * RT, ht].rearrange(
                                "kb p c -> p kb c"
                            ),
                            in_=lt[:].rearrange("p (kb c) -> p kb c", kb=RT),
                        )
                    # V tiles: transpose L.T chunk back to natural layout
                    for kb in range(RT):
                        vt = vp.tile([P, H], BF, tag="v")
                        pt = tps.tile([P, H], BF, tag="tpv")
                        for ht in range(HT):
                            nc.tensor.transpose(
                                pt[:, ht * P : (ht + 1) * P],
                                lts[ht][:, kb * P : (kb + 1) * P],
                                ident[:],
                            )
                        nc.vector.tensor_copy(out=vt[:], in_=pt[:])
                        nc.sync.dma_start(out=v_d[ch * RT + kb], in_=vt[:])

            # ---------------- Phase 2: attention ----------------
            with (
                tc.tile_pool(name="qtp", bufs=HT) as qtp,
                tc.tile_pool(name="op", bufs=QTN) as op,
                tc.tile_pool(name="lkp", bufs=4) as lkp,
                tc.tile_pool(name="ep", bufs=2 * BLK) as ep,
                tc.tile_pool(name="vp2", bufs=2 * BLK) as vp2,
                tc.tile_pool(name="fin", bufs=2) as fin,
                tc.tile_pool(name="sps", bufs=2, space="PSUM") as sps,
                tc.tile_pool(name="ops", bufs=2, space="PSUM") as ops,
                tc.tile_pool(name="cps", bufs=1, space="PSUM") as cps,
            ):
                # Q.T resident: first NL columns of L.T
                qts = []
                for ht in range(HT):
                    q = qtp.tile([P, NL], BF, tag="qt")
                    nc.sync.dma_start(
                        out=q[:].rearrange("p (k c) -> p k c", k=QTN),
                        in_=lt_d[0:QTN, ht].rearrange("k p c -> p k c"),
                    )
                    qts.append(q)

                psum_c = cps.tile([P, QTN], F32, tag="cs")
                out_sb = []
                for qt in range(QTN):
                    out_sb.append(op.tile([P, H], F32, tag="o", name="o"))

                for blk in range(NBLK):
                    es = []
                    vts = []
                    for j in range(BLK):
                        kt = blk * BLK + j
                        ltk = lkp.tile([P, H], BF, tag="lk")
                        nc.sync.dma_start(
                            out=ltk[:].rearrange("p (h c) -> p h c", h=HT),
                            in_=lt_d[kt].rearrange("h p c -> p h c"),
                        )
                        e = ep.tile([P, NL], BF, tag="e")
                        for qc in range(NL // CHUNK):
                            ps = sps.tile([P, CHUNK], F32, tag="sp")
                            for ht in range(HT):
                                nc.tensor.matmul(
                                    ps[:],
                                    lhsT=ltk[:, ht * P : (ht + 1) * P],
                                    rhs=qts[ht][:, qc * CHUNK : (qc + 1) * CHUNK],
                                    start=(ht == 0),
                                    stop=(ht == HT - 1),
                                )
                            nc.scalar.activation(
                                out=e[:, qc * CHUNK : (qc + 1) * CHUNK],
                                in_=ps[:],
                                func=EXP,
                                scale=SCALE,
                            )
                        es.append(e)
                        # colsum accumulation (denominator), one chain per q tile
                        for qt in range(QTN):
                            nc.tensor.matmul(
                                psum_c[:, qt : qt + 1],
                                lhsT=e[:, qt * P : (qt + 1) * P],
                                rhs=ones[:],
                                start=(kt == 0),
                                stop=(kt == KT - 1),
                            )
                        vt = vp2.tile([P, H], BF, tag="v2")
                        nc.sync.dma_start(out=vt[:], in_=v_d[kt])
                        vts.append(vt)

                    last_blk = blk == NBLK - 1
                    if last_blk:
                        # colsum is complete: build its row layout for the
                        # exact rank-1 bias term colsum[q] * b[h]
                        cs_sb = fin.tile([P, QTN], F32, tag="cs_sb")
                        nc.vector.tensor_copy(out=cs_sb[:], in_=psum_c[:])
                        cs_row = fin.tile([1, NL], F32, tag="cs_row")
                        for qt in range(QTN):
                            cs_tp = sps.tile([1, P], F32, tag="ct", bufs=1)
                            nc.tensor.transpose(
                                cs_tp[:], cs_sb[:, qt : qt + 1], ident32[:]
                            )
                            nc.vector.tensor_copy(
                                out=cs_row[0:1, qt * P : (qt + 1) * P], in_=cs_tp[:]
                            )
                    for qt in range(QTN):
                        po = ops.tile([P, H], F32, tag="op")
                        for j in range(BLK):
                            for hh in range(H // CHUNK):
                                nc.tensor.matmul(
                                    po[:, hh * CHUNK : (hh + 1) * CHUNK],
                                    lhsT=es[j][:, qt * P : (qt + 1) * P],
                                    rhs=vts[j][:, hh * CHUNK : (hh + 1) * CHUNK],
                                    start=(j == 0),
                                    stop=(j == BLK - 1 and not last_blk),
                                )
                        if last_blk:
                            for hh in range(H // CHUNK):
                                nc.tensor.matmul(
                                    po[:, hh * CHUNK : (hh + 1) * CHUNK],
                                    lhsT=cs_row[0:1, qt * P : (qt + 1) * P],
                                    rhs=b_row[0:1, hh * CHUNK : (hh + 1) * CHUNK],
                                    start=False,
                                    stop=True,
                                )
                        if blk == 0:
                            nc.vector.tensor_copy(out=out_sb[qt][:], in_=po[:])
                        else:
                            nc.vector.tensor_add(
                                out=out_sb[qt][:], in0=out_sb[qt][:], in1=po[:]
                            )

                rec = fin.tile([P, QTN], F32, tag="rec")
                nc.vector.reciprocal(rec[:], psum_c[:])
                for qt in range(QTN):
                    nc.vector.tensor_mul(
                        out=out_sb[qt][:],
                        in0=out_sb[qt][:],
                        in1=rec[:, qt : qt + 1].to_broadcast([P, H]),
                    )
                    nc.sync.dma_start(
                        out=out[qt * P : (qt + 1) * P, :], in_=out_sb[qt][:]
                    )
    nc.finalize()
    return nc


def _prep_inputs(inputs):
    ids = np.asarray(inputs["input_ids"]).astype(np.int32)
    pids = np.asarray(inputs["pos_ids"]).astype(np.int32)
    emb = np.asarray(inputs["emb"], dtype=np.float32)
    pemb = np.asarray(inputs["pos_emb"], dtype=np.float32)
    W = np.asarray(inputs["W"], dtype=np.float32)
    b = np.asarray(inputs["b"], dtype=np.float32)
    wt = np.ascontiguousarray(W.T)                      # [2H, H]
    bias = np.ascontiguousarray(b.reshape(HT, P, 1))
    in_maps = []
    for i in range(NCORES):
        r = np.roll(ids, -NL * i)
        rp = np.roll(pids, -NL * i)
        in_maps.append(
            {
                "ids": np.ascontiguousarray(r.reshape(KT, P, 1)),
                "pids": np.ascontiguousarray(rp.reshape(KT, P, 1)),
                "emb": emb,
                "pemb": pemb,
                "wt": wt,
                "bias": bias,
            }
        )
    return in_maps


def run(inputs, trace=False):
    nc = build_nc()
    in_maps = _prep_inputs(inputs)
    res = run_bass_kernel_spmd(nc, in_maps, list(range(NCORES)), trace=trace)
    out = np.concatenate([res.results[i]["out"] for i in range(NCORES)], axis=0)
    return out, res


def kernel(**inputs):
    out, _ = run(inputs, trace=False)
    return out



# revision 3
# speedup vs baseline: 1.3955x; 1.3955x over previous
"""Trainium2 Bass kernel for nn_AttentiveEncoderPOS (embed+concat+linear+self-attention).

Strategy (8 cores, sequence-parallel with AllGather):
  - Each core gathers/computes only ITS 1024-row slice of
    L = concat(emb[ids], pos[pids]) @ W.T + b, in transposed layout
    (L.T, h on partitions). Its own slice stays resident in SBUF (qres,
    the queries); an 8-core AllGather shares all slices.
  - Full L.T (16MB bf16) is then loaded resident into SBUF. Scores use L.T
    columns as stationary (keys) and qres as moving (queries) operands; V
    tiles ([keys, h] natural layout) are produced on the fly by PE
    transposes of resident L.T blocks, so phase 2 streams nothing from DRAM.
  - Queries processed in 2 passes of 512 to fit e-tiles/accumulators in
    SBUF alongside the resident L.T. Scores are tiny (|s|<0.01) so exp()
    without max-subtraction is exact softmax; denominator via
    ones-stationary matmul chains per block.
"""

import numpy as np

import concourse.bass as bass
import concourse.mybir as mybir
from concourse import bacc
from concourse.tile import TileContext
from concourse.bass_utils import run_bass_kernel_spmd
from concourse.masks import make_identity

N = 8192
H = 1024
VOCAB = 50257
POS = 64
NCORES = 8
NL = N // NCORES          # 1024 rows (queries) per core
P = 128
HT = H // P               # 8 h tiles
K2 = 2 * H
KTI = K2 // P             # 16 contraction tiles for the linear
RTOT = NL // P            # 8 row tiles per core
CHUNK = 512
NCH = NL // CHUNK         # 2 phase-1 chunks
RT = CHUNK // P           # 4 row tiles / chunk
KT = N // P               # 64 key tiles
BLK = 8                   # key tiles per phase-2 block
NBLK = KT // BLK
QW = 512                  # queries per phase-2 pass
NPASS = NL // QW          # 2 passes
QTP = QW // P             # 4 q tiles per pass
HH = H // CHUNK           # 2 A@V output chunks
SCALE = 1.0 / 32.0        # 1/sqrt(H)

BF = mybir.dt.bfloat16
F32 = mybir.dt.float32
I32 = mybir.dt.int32
EXP = mybir.ActivationFunctionType.Exp


def build_nc():
    nc = bacc.Bacc()
    ids = nc.declare_dram_parameter("ids", [RTOT, P, 1], I32, isOutput=False)
    pids = nc.declare_dram_parameter("pids", [RTOT, P, 1], I32, isOutput=False)
    emb = nc.declare_dram_parameter("emb", [VOCAB, H], F32, isOutput=False)
    pemb = nc.declare_dram_parameter("pemb", [POS, H], F32, isOutput=False)
    wt = nc.declare_dram_parameter("wt", [K2, H], F32, isOutput=False)  # W.T
    bias = nc.declare_dram_parameter("bias", [HT, P, 1], F32, isOutput=False)
    out = nc.declare_dram_parameter("out", [NL, H], F32, isOutput=True)

    # AllGather bounce buffers: own L.T chunk -> all cores' chunks
    ag_in = nc.dram_tensor("ag_in", [HT, P, NL], BF)
    ag_out = nc.dram_tensor("ag_out", [NCORES, HT, P, NL], BF, addr_space="Shared")

    with TileContext(nc) as tc:
        with (
            tc.tile_pool(name="const", bufs=1) as const,
            tc.tile_pool(name="qres", bufs=1) as qresp,
        ):
            ident = const.tile([P, P], BF)
            make_identity(nc, ident[:])
            ones = const.tile([P, 1], BF)
            nc.gpsimd.memset(ones[:], 1.0)
            one1 = const.tile([1, 1], F32)
            nc.gpsimd.memset(one1[:], 1.0)
            b_sb = const.tile([P, HT], F32)
            nc.sync.dma_start(
                out=b_sb[:].rearrange("p (h u) -> p h u", h=HT),
                in_=bias.rearrange("h p u -> p h u"),
            )
            # own L.T chunk, resident across both phases (the queries)
            qres = [
                qresp.tile([P, NL], BF, tag=f"q{ht}", name=f"q{ht}")
                for ht in range(HT)
            ]

            # ---------------- Phase 1: own L.T chunk ----------------
            with (
                tc.tile_pool(name="wld", bufs=2) as wld,
                tc.tile_pool(name="wtp", bufs=1) as wtp,
                tc.tile_pool(name="idp", bufs=8) as idp,
                tc.tile_pool(name="xfp", bufs=3) as xfp,
                tc.tile_pool(name="xbp", bufs=RT + 1) as xbp,
                tc.tile_pool(name="xtp", bufs=KTI + 2) as xtp,
                tc.tile_pool(name="tps", bufs=2, space="PSUM") as tps,
                tc.tile_pool(name="mps", bufs=2, space="PSUM") as mps,
            ):
                # W.T -> bf16 SBUF, one [128, H] tile per contraction k-tile
                wtb = []
                for k in range(KTI):
                    wf = wld.tile([P, H], F32, tag="wld")
                    nc.sync.dma_start(out=wf[:], in_=wt[k * P : (k + 1) * P, :])
                    wb = wtp.tile([P, H], BF, tag=f"wtb{k}", name=f"wtb{k}")
                    nc.vector.tensor_copy(out=wb[:], in_=wf[:])
                    wtb.append(wb)

                for ch in range(NCH):
                    xbs = []
                    for rt in range(RT):
                        t = ch * RT + rt
                        idt = idp.tile([P, 1], I32, tag="id")
                        nc.sync.dma_start(out=idt[:], in_=ids[t])
                        pidt = idp.tile([P, 1], I32, tag="pid")
                        nc.sync.dma_start(out=pidt[:], in_=pids[t])
                        xf = xfp.tile([P, K2], F32, tag="xf")
                        nc.gpsimd.indirect_dma_start(
                            out=xf[:, 0:H],
                            out_offset=None,
                            in_=emb[:],
                            in_offset=bass.IndirectOffsetOnAxis(ap=idt[:, :1], axis=0),
                        )
                        nc.gpsimd.indirect_dma_start(
                            out=xf[:, H:K2],
                            out_offset=None,
                            in_=pemb[:],
                            in_offset=bass.IndirectOffsetOnAxis(ap=pidt[:, :1], axis=0),
                        )
                        xb = xbp.tile([P, K2], BF, tag="xb")
                        nc.vector.tensor_copy(out=xb[:], in_=xf[:])
                        xbs.append(xb)
                    xts = []
                    for k in range(KTI):
                        pt = tps.tile([P, CHUNK], BF, tag="tp")
                        for rt in range(RT):
                            nc.tensor.transpose(
                                pt[:, rt * P : (rt + 1) * P],
                                xbs[rt][:, k * P : (k + 1) * P],
                                ident[:],
                            )
                        xt = xtp.tile([P, CHUNK], BF, tag="xt")
                        nc.vector.tensor_copy(out=xt[:], in_=pt[:])
                        xts.append(xt)

                    # linear: L.T[ht, chunk] = sum_k W.T[k,ht].T @ X.T[k,chunk]
                    for ht in range(HT):
                        pm = mps.tile([P, CHUNK], F32, tag="mp")
                        for k in range(KTI):
                            nc.tensor.matmul(
                                pm[:],
                                lhsT=wtb[k][:, ht * P : (ht + 1) * P],
                                rhs=xts[k][:],
                                start=(k == 0),
                                stop=(k == KTI - 1),
                            )
                        csl = slice(ch * CHUNK, (ch + 1) * CHUNK)
                        nc.vector.tensor_add(
                            out=qres[ht][:, csl],
                            in0=pm[:],
                            in1=b_sb[:, ht : ht + 1].to_broadcast([P, CHUNK]),
                        )
                        nc.sync.dma_start(
                            out=ag_in[ht][:, csl], in_=qres[ht][:, csl]
                        )

            # ---------------- AllGather L.T across the 8 cores ----------------
            nc.gpsimd.collective_compute(
                "AllGather",
                mybir.AluOpType.bypass,
                replica_groups=[list(range(NCORES))],
                ins=[ag_in[:].opt()],
                outs=[ag_out[:].opt()],
            )

            # ---------------- Phase 2: attention ----------------
            with (
                tc.tile_pool(name="ltr", bufs=1) as ltr,
                tc.tile_pool(name="ep", bufs=12) as epool,
                tc.tile_pool(name="vp", bufs=BLK + 1) as vp,
                tc.tile_pool(name="osb", bufs=QTP) as osb,
                tc.tile_pool(name="fin", bufs=4) as fin,
                tc.tile_pool(name="sps", bufs=2, space="PSUM") as sps,
                tc.tile_pool(name="ops", bufs=2, space="PSUM") as ops,
                tc.tile_pool(name="tvs", bufs=1, space="PSUM") as tvs,
                tc.tile_pool(name="cps", bufs=1, space="PSUM") as cps,
            ):
                # Full L.T resident: 8 tiles [128 h, 8192 keys] bf16 (16MB)
                lt_res = []
                for ht in range(HT):
                    lr = ltr.tile([P, N], BF, tag=f"ltr{ht}", name=f"ltr{ht}")
                    nc.sync.dma_start(
                        out=lr[:].rearrange("p (c n) -> p c n", c=NCORES),
                        in_=ag_out[:, ht].rearrange("c p n -> p c n"),
                    )
                    lt_res.append(lr)

                for pss in range(NPASS):
                    qsl = slice(pss * QW, (pss + 1) * QW)
                    csum = fin.tile([1, QW], F32, tag="csum")
                    out_sb = [
                        osb.tile([P, H], F32, tag="o", name="o") for _ in range(QTP)
                    ]
                    for blk in range(NBLK):
                        es = []
                        vts = []
                        psc = cps.tile([1, QW], F32, tag="cs")
                        for j in range(BLK):
                            kt = blk * BLK + j
                            ksl = slice(kt * P, (kt + 1) * P)
                            ps = sps.tile([P, QW], F32, tag="sp")
                            for ht in range(HT):
                                nc.tensor.matmul(
                                    ps[:],
                                    lhsT=lt_res[ht][:, ksl],
                                    rhs=qres[ht][:, qsl],
                                    start=(ht == 0),
                                    stop=(ht == HT - 1),
                                )
                            e = epool.tile([P, QW], BF, tag="e")
                            nc.scalar.activation(
                                out=e[:], in_=ps[:], func=EXP, scale=SCALE
                            )
                            es.append(e)
                            # denominator partial: colsum over this key tile
                            nc.tensor.matmul(
                                psc[:],
                                lhsT=ones[:],
                                rhs=e[:],
                                start=(j == 0),
                                stop=(j == BLK - 1),
                            )
                            # V tile: transpose resident L.T block to [keys, h]
                            pt = tvs.tile([P, H], BF, tag="tv")
                            for ht in range(HT):
                                nc.tensor.transpose(
                                    pt[:, ht * P : (ht + 1) * P],
                                    lt_res[ht][:, ksl],
                                    ident[:],
                                )
                            vt = vp.tile([P, H], BF, tag="v")
                            nc.vector.tensor_copy(out=vt[:], in_=pt[:])
                            vts.append(vt)
                        if blk == 0:
                            nc.vector.tensor_copy(out=csum[:], in_=psc[:])
                        else:
                            nc.vector.tensor_add(out=csum[:], in0=csum[:], in1=psc[:])
                        for qt in range(QTP):
                            po = ops.tile([P, H], F32, tag="op")
                            for j in range(BLK):
                                for hh in range(HH):
                                    nc.tensor.matmul(
                                        po[:, hh * CHUNK : (hh + 1) * CHUNK],
                                        lhsT=es[j][:, qt * P : (qt + 1) * P],
                                        rhs=vts[j][:, hh * CHUNK : (hh + 1) * CHUNK],
                                        start=(j == 0),
                                        stop=(j == BLK - 1),
                                    )
                            if blk == 0:
                                nc.vector.tensor_copy(out=out_sb[qt][:], in_=po[:])
                            else:
                                nc.vector.tensor_add(
                                    out=out_sb[qt][:], in0=out_sb[qt][:], in1=po[:]
                                )
                    # normalize: rec = 1/colsum, move q to partitions, scale, store
                    rec_row = fin.tile([1, QW], F32, tag="rr")
                    nc.vector.reciprocal(rec_row[:], csum[:])
                    for qt in range(QTP):
                        ct = cps.tile([P, 1], F32, tag="cs")
                        nc.tensor.matmul(
                            ct[:],
                            lhsT=rec_row[0:1, qt * P : (qt + 1) * P],
                            rhs=one1[0:1, 0:1],
                            start=True,
                            stop=True,
                        )
                        rec = fin.tile([P, 1], F32, tag="rec")
                        nc.vector.tensor_copy(out=rec[:], in_=ct[:])
                        nc.vector.tensor_mul(
                            out=out_sb[qt][:],
                            in0=out_sb[qt][:],
                            in1=rec[:, 0:1].to_broadcast([P, H]),
                        )
                        nc.sync.dma_start(
                            out=out[pss * QW + qt * P : pss * QW + (qt + 1) * P, :],
                            in_=out_sb[qt][:],
                        )
    nc.finalize()
    return nc


def _prep_inputs(inputs):
    ids = np.asarray(inputs["input_ids"]).astype(np.int32)
    pids = np.asarray(inputs["pos_ids"]).astype(np.int32)
    emb = np.asarray(inputs["emb"], dtype=np.float32)
    pemb = np.asarray(inputs["pos_emb"], dtype=np.float32)
    W = np.asarray(inputs["W"], dtype=np.float32)
    b = np.asarray(inputs["b"], dtype=np.float32)
    wt = np.ascontiguousarray(W.T)                      # [2H, H]
    bias = np.ascontiguousarray(b.reshape(HT, P, 1))
    in_maps = []
    for i in range(NCORES):
        sl = slice(i * NL, (i + 1) * NL)
        in_maps.append(
            {
                "ids": np.ascontiguousarray(ids[sl].reshape(RTOT, P, 1)),
                "pids": np.ascontiguousarray(pids[sl].reshape(RTOT, P, 1)),
                "emb": emb,
                "pemb": pemb,
                "wt": wt,
                "bias": bias,
            }
        )
    return in_maps


def run(inputs, trace=False):
    nc = build_nc()
    in_maps = _prep_inputs(inputs)
    res = run_bass_kernel_spmd(nc, in_maps, list(range(NCORES)), trace=trace)
    out = np.concatenate([res.results[i]["out"] for i in range(NCORES)], axis=0)
    return out, res


def kernel(**inputs):
    out, _ = run(inputs, trace=False)
    return out


# revision 7
# speedup vs baseline: 1.5003x; 1.0751x over previous
"""Trainium2 Bass kernel for nn_AttentiveEncoderPOS (embed+concat+linear+self-attention).

Strategy (8 cores, sequence-parallel with AllGather):
  - Each core gathers/computes only ITS 1024-row slice of
    L = concat(emb[ids], pos[pids]) @ W.T + b, in transposed layout
    (L.T, h on partitions). Its own slice stays resident in SBUF (qres,
    the queries); an 8-core AllGather shares all slices.
  - Full L.T (16MB bf16) is then loaded resident into SBUF. Scores use L.T
    columns as stationary (keys) and qres as moving (queries) operands; V
    tiles ([keys, h] natural layout) are produced on the fly by PE
    transposes of resident L.T blocks, so phase 2 streams nothing from DRAM.
  - Queries processed in 2 passes of 512 to fit e-tiles/accumulators in
    SBUF alongside the resident L.T. Scores are tiny (|s|<0.01) so exp()
    without max-subtraction is exact softmax; denominator via
    ones-stationary matmul chains per block.
"""

import numpy as np

import concourse.bass as bass
import concourse.mybir as mybir
from concourse import bacc
from concourse.tile import TileContext
from concourse.bass_utils import run_bass_kernel_spmd
from concourse.masks import make_identity

N = 8192
H = 1024
VOCAB = 50257
POS = 64
NCORES = 8
NL = N // NCORES          # 1024 rows (queries) per core
P = 128
HT = H // P               # 8 h tiles
K2 = 2 * H
KTI = K2 // P             # 16 contraction tiles for the linear
RTOT = NL // P            # 8 row tiles per core
CHUNK = 512
NCH = NL // CHUNK         # 2 phase-1 chunks
RT = CHUNK // P           # 4 row tiles / chunk
KT = N // P               # 64 key tiles
BLK = 8                   # key tiles per phase-2 block
NBLK = KT // BLK
QW = 512                  # queries per phase-2 pass
NPASS = NL // QW          # 2 passes
QTP = QW // P             # 4 q tiles per pass
HH = H // CHUNK           # 2 A@V output chunks
SCALE = 1.0 / 32.0        # 1/sqrt(H)

BF = mybir.dt.bfloat16
F32 = mybir.dt.float32
I32 = mybir.dt.int32
EXP = mybir.ActivationFunctionType.Exp


def build_nc():
    nc = bacc.Bacc()
    ids = nc.declare_dram_parameter("ids", [RTOT, P, 1], I32, isOutput=False)
    pids = nc.declare_dram_parameter("pids", [RTOT, P, 1], I32, isOutput=False)
    emb = nc.declare_dram_parameter("emb", [VOCAB, H], F32, isOutput=False)
    pemb = nc.declare_dram_parameter("pemb", [POS, H], F32, isOutput=False)
    wt = nc.declare_dram_parameter("wt", [K2, H], F32, isOutput=False)  # W.T
    bias = nc.declare_dram_parameter("bias", [HT, P, 1], F32, isOutput=False)
    out = nc.declare_dram_parameter("out", [NL, H], F32, isOutput=True)

    # AllGather bounce buffers, one pair per 512-row phase-1 chunk so the
    # gather of chunk 0 overlaps phase-1 compute of chunk 1 and phase 2.
    ag_in = [nc.dram_tensor(f"ag_in{c}", [HT, P, CHUNK], BF) for c in range(NCH)]
    ag_out = [
        nc.dram_tensor(
            f"ag_out{c}", [NCORES, HT, P, CHUNK], BF, addr_space="Shared"
        )
        for c in range(NCH)
    ]

    with TileContext(nc) as tc:
        with (
            tc.tile_pool(name="const", bufs=1) as const,
            tc.tile_pool(name="qres", bufs=1) as qresp,
        ):
            ident = const.tile([P, P], BF)
            make_identity(nc, ident[:])
            ones32 = const.tile([P, 1], F32)
            nc.gpsimd.memset(ones32[:], 1.0)
            one1 = const.tile([1, 1], F32)
            nc.gpsimd.memset(one1[:], 1.0)
            b_sb = const.tile([P, HT], F32)
            nc.sync.dma_start(
                out=b_sb[:].rearrange("p (h u) -> p h u", h=HT),
                in_=bias.rearrange("h p u -> p h u"),
            )
            # own L.T chunk, resident across both phases (the queries)
            qres = [
                qresp.tile([P, NL], BF, tag=f"q{ht}", name=f"q{ht}")
                for ht in range(HT)
            ]

            # ---------------- Phase 1: own L.T chunk ----------------
            with (
                tc.tile_pool(name="wld", bufs=2) as wld,
                tc.tile_pool(name="wtp", bufs=1) as wtp,
                tc.tile_pool(name="idp", bufs=8) as idp,
                tc.tile_pool(name="xfp", bufs=3) as xfp,
                tc.tile_pool(name="xbp", bufs=RT + 1) as xbp,
                tc.tile_pool(name="xtp", bufs=KTI + 2) as xtp,
                tc.tile_pool(name="tps", bufs=2, space="PSUM") as tps,
                tc.tile_pool(name="mps", bufs=2, space="PSUM") as mps,
            ):
                # W.T -> bf16 SBUF, one [128, H] tile per contraction k-tile
                wtb = []
                for k in range(KTI):
                    wf = wld.tile([P, H], F32, tag="wld")
                    nc.sync.dma_start(out=wf[:], in_=wt[k * P : (k + 1) * P, :])
                    wb = wtp.tile([P, H], BF, tag=f"wtb{k}", name=f"wtb{k}")
                    nc.vector.tensor_copy(out=wb[:], in_=wf[:])
                    wtb.append(wb)

                for ch in range(NCH):
                    xbs = []
                    for rt in range(RT):
                        t = ch * RT + rt
                        idt = idp.tile([P, 1], I32, tag="id")
                        nc.sync.dma_start(out=idt[:], in_=ids[t])
                        pidt = idp.tile([P, 1], I32, tag="pid")
                        nc.sync.dma_start(out=pidt[:], in_=pids[t])
                        xf = xfp.tile([P, K2], F32, tag="xf")
                        nc.gpsimd.indirect_dma_start(
                            out=xf[:, 0:H],
                            out_offset=None,
                            in_=emb[:],
                            in_offset=bass.IndirectOffsetOnAxis(ap=idt[:, :1], axis=0),
                        )
                        nc.gpsimd.indirect_dma_start(
                            out=xf[:, H:K2],
                            out_offset=None,
                            in_=pemb[:],
                            in_offset=bass.IndirectOffsetOnAxis(ap=pidt[:, :1], axis=0),
                        )
                        xb = xbp.tile([P, K2], BF, tag="xb")
                        nc.vector.tensor_copy(out=xb[:], in_=xf[:])
                        xbs.append(xb)
                    xts = []
                    for k in range(KTI):
                        pt = tps.tile([P, CHUNK], BF, tag="tp")
                        for rt in range(RT):
                            nc.tensor.transpose(
                                pt[:, rt * P : (rt + 1) * P],
                                xbs[rt][:, k * P : (k + 1) * P],
                                ident[:],
                            )
                        xt = xtp.tile([P, CHUNK], BF, tag="xt")
                        nc.vector.tensor_copy(out=xt[:], in_=pt[:])
                        xts.append(xt)

                    # linear: L.T[ht, chunk] = sum_k W.T[k,ht].T @ X.T[k,chunk]
                    for ht in range(HT):
                        pm = mps.tile([P, CHUNK], F32, tag="mp")
                        for k in range(KTI):
                            nc.tensor.matmul(
                                pm[:],
                                lhsT=wtb[k][:, ht * P : (ht + 1) * P],
                                rhs=xts[k][:],
                                start=(k == 0),
                                stop=(k == KTI - 1),
                            )
                        csl = slice(ch * CHUNK, (ch + 1) * CHUNK)
                        nc.vector.tensor_add(
                            out=qres[ht][:, csl],
                            in0=pm[:],
                            in1=b_sb[:, ht : ht + 1].to_broadcast([P, CHUNK]),
                        )
                        nc.sync.dma_start(
                            out=ag_in[ch][ht], in_=qres[ht][:, csl]
                        )
                    # AllGather this 512-row chunk while the next computes
                    nc.gpsimd.collective_compute(
                        "AllGather",
                        mybir.AluOpType.bypass,
                        replica_groups=[list(range(NCORES))],
                        ins=[ag_in[ch][:].opt()],
                        outs=[ag_out[ch][:].opt()],
                    )

            # ---------------- Phase 2: attention ----------------
            with (
                tc.tile_pool(name="ltr", bufs=1) as ltr,
                tc.tile_pool(name="ep", bufs=12) as epool,
                tc.tile_pool(name="vp", bufs=BLK + 1) as vp,
                tc.tile_pool(name="osb", bufs=QTP) as osb,
                tc.tile_pool(name="fin", bufs=4) as fin,
                tc.tile_pool(name="sps", bufs=2, space="PSUM") as sps,
                tc.tile_pool(name="ops", bufs=2, space="PSUM") as ops,
                tc.tile_pool(name="tvs", bufs=1, space="PSUM") as tvs,
                tc.tile_pool(name="cps", bufs=1, space="PSUM") as cps,
            ):
                # Full L.T resident: 8 tiles [128 h, 8192 keys] bf16 (16MB),
                # loaded per AllGather chunk so reads pipeline behind the AGs.
                lt_res = [
                    ltr.tile([P, N], BF, tag=f"ltr{ht}", name=f"ltr{ht}")
                    for ht in range(HT)
                ]
                for ch in range(NCH):
                    for ht in range(HT):
                        dst = lt_res[ht][:].rearrange(
                            "p (c g n) -> p c g n", c=NCORES, g=NCH
                        )[:, :, ch, :]
                        nc.sync.dma_start(
                            out=dst,
                            in_=ag_out[ch][:, ht].rearrange("c p n -> p c n"),
                        )

                # key-tile blocks ordered chunk-half first, so the first half
                # only depends on AllGather 0
                blocks = [
                    [c8 * RTOT + half * RT + jj for c8 in (2 * bc, 2 * bc + 1)
                     for jj in range(RT)]
                    for half in range(NCH)
                    for bc in range(NCORES // 2)
                ]

                for pss in range(NPASS):
                    qsl = slice(pss * QW, (pss + 1) * QW)
                    csum = fin.tile([P, QW], F32, tag="csum")
                    out_sb = [
                        osb.tile([P, H], F32, tag="o", name="o") for _ in range(QTP)
                    ]
                    for blk, kts in enumerate(blocks):
                        es = []
                        vts = []
                        for j, kt in enumerate(kts):
                            ksl = slice(kt * P, (kt + 1) * P)
                            ps = sps.tile([P, QW], F32, tag="sp")
                            for ht in range(HT):
                                nc.tensor.matmul(
                                    ps[:],
                                    lhsT=lt_res[ht][:, ksl],
                                    rhs=qres[ht][:, qsl],
                                    start=(ht == 0),
                                    stop=(ht == HT - 1),
                                )
                            e = epool.tile([P, QW], BF, tag="e")
                            nc.scalar.activation(
                                out=e[:], in_=ps[:], func=EXP, scale=SCALE
                            )
                            es.append(e)
                            # denominator partial (cross-partition sum at end)
                            if blk == 0 and j == 0:
                                nc.vector.tensor_copy(out=csum[:], in_=e[:])
                            else:
                                nc.vector.tensor_add(
                                    out=csum[:], in0=csum[:], in1=e[:]
                                )
                            # V tile: transpose resident L.T block to [keys, h]
                            pt = tvs.tile([P, H], BF, tag="tv")
                            for ht in range(HT):
                                nc.tensor.transpose(
                                    pt[:, ht * P : (ht + 1) * P],
                                    lt_res[ht][:, ksl],
                                    ident[:],
                                )
                            vt = vp.tile([P, H], BF, tag="v")
                            nc.vector.tensor_copy(out=vt[:], in_=pt[:])
                            vts.append(vt)
                        for qt in range(QTP):
                            po = ops.tile([P, H], F32, tag="op")
                            for j in range(BLK):
                                for hh in range(HH):
                                    nc.tensor.matmul(
                                        po[:, hh * CHUNK : (hh + 1) * CHUNK],
                                        lhsT=es[j][:, qt * P : (qt + 1) * P],
                                        rhs=vts[j][:, hh * CHUNK : (hh + 1) * CHUNK],
                                        start=(j == 0),
                                        stop=(j == BLK - 1),
                                    )
                            if blk == 0:
                                nc.vector.tensor_copy(out=out_sb[qt][:], in_=po[:])
                            else:
                                nc.vector.tensor_add(
                                    out=out_sb[qt][:], in0=out_sb[qt][:], in1=po[:]
                                )
                    # normalize: colsum -> reciprocal row, move q to partitions
                    psc = cps.tile([1, QW], F32, tag="cs")
                    nc.tensor.matmul(
                        psc[:], lhsT=ones32[:], rhs=csum[:], start=True, stop=True
                    )
                    rec_row = fin.tile([1, QW], F32, tag="rr")
                    nc.vector.reciprocal(rec_row[:], psc[:])
                    for qt in range(QTP):
                        ct = cps.tile([P, 1], F32, tag="cs")
                        nc.tensor.matmul(
                            ct[:],
                            lhsT=rec_row[0:1, qt * P : (qt + 1) * P],
                            rhs=one1[0:1, 0:1],
                            start=True,
                            stop=True,
                        )
                        rec = fin.tile([P, 1], F32, tag="rec")
                        nc.vector.tensor_copy(out=rec[:], in_=ct[:])
                        nc.vector.tensor_mul(
                            out=out_sb[qt][:],
                            in0=out_sb[qt][:],
                            in1=rec[:, 0:1].to_broadcast([P, H]),
                        )
                        nc.sync.dma_start(
                            out=out[pss * QW + qt * P : pss * QW + (qt + 1) * P, :],
                            in_=out_sb[qt][:],
                        )
    nc.finalize()
    return nc


def _prep_inputs(inputs):
    ids = np.asarray(inputs["input_ids"]).astype(np.int32)
    pids = np.asarray(inputs["pos_ids"]).astype(np.int32)
    emb = np.asarray(inputs["emb"], dtype=np.float32)
    pemb = np.asarray(inputs["pos_emb"], dtype=np.float32)
    W = np.asarray(inputs["W"], dtype=np.float32)
    b = np.asarray(inputs["b"], dtype=np.float32)
    wt = np.ascontiguousarray(W.T)                      # [2H, H]
    bias = np.ascontiguousarray(b.reshape(HT, P, 1))
    in_maps = []
    for i in range(NCORES):
        sl = slice(i * NL, (i + 1) * NL)
        in_maps.append(
            {
                "ids": np.ascontiguousarray(ids[sl].reshape(RTOT, P, 1)),
                "pids": np.ascontiguousarray(pids[sl].reshape(RTOT, P, 1)),
                "emb": emb,
                "pemb": pemb,
                "wt": wt,
                "bias": bias,
            }
        )
    return in_maps


def run(inputs, trace=False):
    nc = build_nc()
    in_maps = _prep_inputs(inputs)
    res = run_bass_kernel_spmd(nc, in_maps, list(range(NCORES)), trace=trace)
    out = np.concatenate([res.results[i]["out"] for i in range(NCORES)], axis=0)
    return out, res


def kernel(**inputs):
    out, _ = run(inputs, trace=False)
    return out


# revision 12
# speedup vs baseline: 2.0338x; 1.3556x over previous
"""Trainium2 Bass kernel for nn_AttentiveEncoderPOS (embed+concat+linear+self-attention).

Strategy (8 cores, sequence-parallel with AllGather):
  - Each core gathers/computes only ITS 1024-row slice of
    L = concat(emb[ids], pos[pids]) @ W.T + b, in transposed layout
    (L.T, h on partitions), quantized to fp8 (x32 scale). A per-chunk
    8-core AllGather (fp8 payload) shares all slices while compute runs.
  - Full fp8 L.T stays resident in SBUF in DoubleRow layout [128, 2, N]
    (adjacent h-tile pairs stacked), so score matmuls run fp8 DoubleRow
    (2 contraction rows/cycle). V tiles are transposed out of the same
    resident fp8 L.T on the PE and widened to bf16; exp() output is bf16,
    so attn @ V runs bf16. Phase 2 streams nothing from DRAM.
  - Scores are tiny (|s|<0.025) so exp() without max-subtraction is exact
    softmax; denominator accumulates on the vector engine.
"""

import numpy as np

import concourse.bass as bass
import concourse.mybir as mybir
from concourse import bacc
from concourse.tile import TileContext
from concourse.bass_utils import run_bass_kernel_spmd
from concourse.masks import make_identity

N = 8192
H = 1024
VOCAB = 50257
POS = 64
NCORES = 8
NL = N // NCORES          # 1024 rows (queries) per core
P = 128
HT = H // P               # 8 h tiles
HT2 = HT // 2             # 4 DoubleRow h-pair tiles
K2 = 2 * H
KTI = K2 // P             # 16 contraction tiles for the linear
RTOT = NL // P            # 8 row tiles per core
CHUNK = 512
NCH = NL // CHUNK         # 2 phase-1 chunks
RT = CHUNK // P           # 4 row tiles / chunk
KT = N // P               # 64 key tiles
BLK = 8                   # key tiles per phase-2 block
NBLK = KT // BLK
QTP = NL // P             # 8 q tiles
QH = NL // CHUNK          # 2 score chunks along q
HH = H // CHUNK           # 2 A@V output chunks
FSCALE = 32.0             # fp8 quantization scale for L
SCALE = 1.0 / 32.0        # 1/sqrt(H)
SCALE8 = SCALE / (FSCALE * FSCALE)

BF = mybir.dt.bfloat16
F8 = mybir.dt.float8e4
F32 = mybir.dt.float32
I32 = mybir.dt.int32
EXP = mybir.ActivationFunctionType.Exp
COPY = mybir.ActivationFunctionType.Copy
DR = mybir.MatmulPerfMode.DoubleRow


def build_nc():
    nc = bacc.Bacc()
    ids = nc.declare_dram_parameter("ids", [RTOT, P, 1], I32, isOutput=False)
    pids = nc.declare_dram_parameter("pids", [RTOT, P, 1], I32, isOutput=False)
    emb = nc.declare_dram_parameter("emb", [VOCAB, H], F32, isOutput=False)
    pemb = nc.declare_dram_parameter("pemb", [POS, H], F32, isOutput=False)
    wt = nc.declare_dram_parameter("wt", [K2, H], F32, isOutput=False)  # W.T
    bias = nc.declare_dram_parameter("bias", [HT, P, 1], F32, isOutput=False)
    out = nc.declare_dram_parameter("out", [NL, H], F32, isOutput=True)

    # AllGather bounce buffers (fp8), one pair per 512-row phase-1 chunk so
    # the gather of chunk 0 overlaps phase-1 compute of chunk 1 and phase 2.
    ag_in = [nc.dram_tensor(f"ag_in{c}", [HT, P, CHUNK], F8) for c in range(NCH)]
    ag_out = [
        nc.dram_tensor(
            f"ag_out{c}", [NCORES, HT, P, CHUNK], F8, addr_space="Shared"
        )
        for c in range(NCH)
    ]

    with TileContext(nc) as tc:
        with (
            tc.tile_pool(name="const", bufs=1) as const,
            tc.tile_pool(name="qres", bufs=1) as qresp,
        ):
            ident8 = const.tile([P, P], F8)
            make_identity(nc, ident8[:])
            identb = const.tile([P, P], BF)
            make_identity(nc, identb[:])
            ones32 = const.tile([P, 1], F32)
            nc.gpsimd.memset(ones32[:], 1.0)
            one1 = const.tile([1, 1], F32)
            nc.gpsimd.memset(one1[:], 1.0 / FSCALE)
            b_sb = const.tile([P, HT], F32)
            nc.sync.dma_start(
                out=b_sb[:].rearrange("p (h u) -> p h u", h=HT),
                in_=bias.rearrange("h p u -> p h u"),
            )
            b32_sb = const.tile([P, HT], F32)
            nc.vector.tensor_scalar_mul(out=b32_sb[:], in0=b_sb[:], scalar1=FSCALE)
            # own fp8 L.T chunk in DoubleRow layout (these are the queries)
            q8 = [
                qresp.tile([P, 2, NL], F8, tag=f"q{h2}", name=f"q{h2}")
                for h2 in range(HT2)
            ]

            # ---------------- Phase 1: own L.T chunk ----------------
            with (
                tc.tile_pool(name="wld", bufs=2) as wld,
                tc.tile_pool(name="wtp", bufs=1) as wtp,
                tc.tile_pool(name="idp", bufs=8) as idp,
                tc.tile_pool(name="xfp", bufs=3) as xfp,
                tc.tile_pool(name="xbp", bufs=RT + 1) as xbp,
                tc.tile_pool(name="xtp", bufs=KTI + 2) as xtp,
                tc.tile_pool(name="tps", bufs=2, space="PSUM") as tps,
                tc.tile_pool(name="mps", bufs=2, space="PSUM") as mps,
            ):
                # ids first, then chunk-0 gathers, so the W loads (on the
                # scalar queue) don't gate the first AllGather.
                idts, pidts = [], []
                for t in range(RTOT):
                    idt = idp.tile([P, 1], I32, tag="id")
                    nc.sync.dma_start(out=idt[:], in_=ids[t])
                    pidt = idp.tile([P, 1], I32, tag="pid")
                    nc.sync.dma_start(out=pidt[:], in_=pids[t])
                    idts.append(idt)
                    pidts.append(pidt)

                wtb = []
                for k in range(KTI):
                    wf = wld.tile([P, H], F32, tag="wld")
                    nc.scalar.dma_start(out=wf[:], in_=wt[k * P : (k + 1) * P, :])
                    wb = wtp.tile([P, H], BF, tag=f"wtb{k}", name=f"wtb{k}")
                    nc.vector.tensor_copy(out=wb[:], in_=wf[:])
                    wtb.append(wb)

                for ch in range(NCH):
                    xbs = []
                    for rt in range(RT):
                        t = ch * RT + rt
                        xf = xfp.tile([P, K2], F32, tag="xf")
                        nc.gpsimd.indirect_dma_start(
                            out=xf[:, 0:H],
                            out_offset=None,
                            in_=emb[:],
                            in_offset=bass.IndirectOffsetOnAxis(
                                ap=idts[t][:, :1], axis=0
                            ),
                        )
                        nc.gpsimd.indirect_dma_start(
                            out=xf[:, H:K2],
                            out_offset=None,
                            in_=pemb[:],
                            in_offset=bass.IndirectOffsetOnAxis(
                                ap=pidts[t][:, :1], axis=0
                            ),
                        )
                        xb = xbp.tile([P, K2], BF, tag="xb")
                        nc.vector.tensor_copy(out=xb[:], in_=xf[:])
                        xbs.append(xb)
                    xts = []
                    for k in range(KTI):
                        pt = tps.tile([P, CHUNK], BF, tag="tp")
                        for rt in range(RT):
                            nc.tensor.transpose(
                                pt[:, rt * P : (rt + 1) * P],
                                xbs[rt][:, k * P : (k + 1) * P],
                                identb[:],
                            )
                        xt = xtp.tile([P, CHUNK], BF, tag="xt")
                        nc.vector.tensor_copy(out=xt[:], in_=pt[:])
                        xts.append(xt)

                    # linear: L.T[ht, chunk] = sum_k W.T[k,ht].T @ X.T[k,chunk]
                    csl = slice(ch * CHUNK, (ch + 1) * CHUNK)
                    for ht in range(HT):
                        pm = mps.tile([P, CHUNK], F32, tag="mp")
                        for k in range(KTI):
                            nc.tensor.matmul(
                                pm[:],
                                lhsT=wtb[k][:, ht * P : (ht + 1) * P],
                                rhs=xts[k][:],
                                start=(k == 0),
                                stop=(k == KTI - 1),
                            )
                        # fp8 quantize: q8 = 32*(pm + b)
                        nc.vector.tensor_scalar(
                            out=q8[ht // 2][:, ht % 2, csl],
                            in0=pm[:],
                            scalar1=b_sb[:, ht : ht + 1],
                            scalar2=FSCALE,
                            op0=mybir.AluOpType.add,
                            op1=mybir.AluOpType.mult,
                        )
                        nc.sync.dma_start(
                            out=ag_in[ch][ht], in_=q8[ht // 2][:, ht % 2, csl]
                        )
                    # AllGather this 512-row chunk while the next computes
                    nc.gpsimd.collective_compute(
                        "AllGather",
                        mybir.AluOpType.bypass,
                        replica_groups=[list(range(NCORES))],
                        ins=[ag_in[ch][:].opt()],
                        outs=[ag_out[ch][:].opt()],
                    )

            # ---------------- Phase 2: attention ----------------
            with (
                tc.tile_pool(name="ltr", bufs=1) as ltr,
                tc.tile_pool(name="ep", bufs=12) as epool,
                tc.tile_pool(name="vp", bufs=BLK + 2) as vp,
                tc.tile_pool(name="osb", bufs=QTP) as osb,
                tc.tile_pool(name="fin", bufs=2) as fin,
                tc.tile_pool(name="sps", bufs=3, space="PSUM") as sps,
                tc.tile_pool(name="ops", bufs=2, space="PSUM") as ops,
                tc.tile_pool(name="tvs", bufs=1, space="PSUM") as tvs,
            ):
                # Full fp8 L.T resident in DoubleRow layout: 4 tiles
                # [128, 2, 8192] (8MB), loaded per AllGather chunk.
                lt8 = [
                    ltr.tile([P, 2, N], F8, tag=f"lt{h2}", name=f"lt{h2}")
                    for h2 in range(HT2)
                ]
                for ch in range(NCH):
                    for ht in range(HT):
                        dst = lt8[ht // 2][:, ht % 2, :].rearrange(
                            "p (c g n) -> p c g n", c=NCORES, g=NCH
                        )[:, :, ch, :]
                        nc.sync.dma_start(
                            out=dst,
                            in_=ag_out[ch][:, ht].rearrange("c p n -> p c n"),
                        )

                # key-tile blocks ordered chunk-half first, so the first half
                # only depends on AllGather 0
                blocks = [
                    [c8 * RTOT + half * RT + jj for c8 in (2 * bc, 2 * bc + 1)
                     for jj in range(RT)]
                    for half in range(NCH)
                    for bc in range(NCORES // 2)
                ]

                csum = fin.tile([P, NL], F32, tag="csum")
                out_sb = [
                    osb.tile([P, H], F32, tag="o", name="o") for _ in range(QTP)
                ]
                for blk, kts in enumerate(blocks):
                    es = []
                    vts = []
                    for j, kt in enumerate(kts):
                        ksl = slice(kt * P, (kt + 1) * P)
                        e = epool.tile([P, NL], BF, tag="e")
                        for qh in range(QH):
                            qsl = slice(qh * CHUNK, (qh + 1) * CHUNK)
                            ps = sps.tile([P, CHUNK], F32, tag="sp")
                            for h2 in range(HT2):
                                nc.tensor.matmul(
                                    ps[:],
                                    lhsT=lt8[h2][:, :, ksl],
                                    rhs=q8[h2][:, :, qsl],
                                    start=(h2 == 0),
                                    stop=(h2 == HT2 - 1),
                                    perf_mode=DR,
                                )
                            nc.scalar.activation(
                                out=e[:, qsl], in_=ps[:], func=EXP, scale=SCALE8
                            )
                        es.append(e)
                        # denominator partial (cross-partition sum at end)
                        if blk == 0 and j == 0:
                            nc.vector.tensor_copy(out=csum[:], in_=e[:])
                        else:
                            nc.vector.tensor_add(out=csum[:], in0=csum[:], in1=e[:])
                        # V tile: transpose resident fp8 L.T block to [keys, h].
                        # fp8 transpose requires output element step 2, so the
                        # PSUM tile carries a dummy trailing dim.
                        pt = tvs.tile([P, H, 2], F8, tag="tv")
                        for ht in range(HT):
                            nc.tensor.transpose(
                                pt[:, ht * P : (ht + 1) * P, 0],
                                lt8[ht // 2][:, ht % 2, ksl],
                                ident8[:],
                            )
                        vt = vp.tile([P, H], BF, tag="v")
                        nc.vector.tensor_copy(out=vt[:], in_=pt[:, :, 0])
                        vts.append(vt)
                    for qt in range(QTP):
                        po = ops.tile([P, H], F32, tag="op")
                        for j in range(BLK):
                            for hh in range(HH):
                                nc.tensor.matmul(
                                    po[:, hh * CHUNK : (hh + 1) * CHUNK],
                                    lhsT=es[j][:, qt * P : (qt + 1) * P],
                                    rhs=vts[j][:, hh * CHUNK : (hh + 1) * CHUNK],
                                    start=(j == 0),
                                    stop=(j == BLK - 1),
                                )
                        if blk == 0:
                            nc.vector.tensor_copy(out=out_sb[qt][:], in_=po[:])
                        else:
                            nc.vector.tensor_add(
                                out=out_sb[qt][:], in0=out_sb[qt][:], in1=po[:]
                            )
                # normalize: colsum -> reciprocal row, move q to partitions.
                # V carried a x32 scale (one1 = 1/32 folds it back in).
                rec_row = fin.tile([1, NL], F32, tag="rr")
                for qh in range(QH):
                    qsl = slice(qh * CHUNK, (qh + 1) * CHUNK)
                    psc = sps.tile([1, CHUNK], F32, tag="sp")
                    nc.tensor.matmul(
                        psc[:], lhsT=ones32[:], rhs=csum[:, qsl],
                        start=True, stop=True,
                    )
                    nc.vector.reciprocal(rec_row[0:1, qsl], psc[:])
                for qt in range(QTP):
                    ct = tvs.tile([P, 1], F32, tag="tv")
                    nc.tensor.matmul(
                        ct[:],
                        lhsT=rec_row[0:1, qt * P : (qt + 1) * P],
                        rhs=one1[0:1, 0:1],
                        start=True,
                        stop=True,
                    )
                    rec = fin.tile([P, 1], F32, tag="rec")
                    nc.vector.tensor_copy(out=rec[:], in_=ct[:])
                    nc.vector.tensor_mul(
                        out=out_sb[qt][:],
                        in0=out_sb[qt][:],
                        in1=rec[:, 0:1].to_broadcast([P, H]),
                    )
                    nc.sync.dma_start(
                        out=out[qt * P : (qt + 1) * P, :], in_=out_sb[qt][:]
                    )
    nc.finalize()
    return nc


def _prep_inputs(inputs):
    ids = np.asarray(inputs["input_ids"]).astype(np.int32)
    pids = np.asarray(inputs["pos_ids"]).astype(np.int32)
    emb = np.asarray(inputs["emb"], dtype=np.float32)
    pemb = np.asarray(inputs["pos_emb"], dtype=np.float32)
    W = np.asarray(inputs["W"], dtype=np.float32)
    b = np.asarray(inputs["b"], dtype=np.float32)
    wt = np.ascontiguousarray(W.T)                      # [2H, H]
    bias = np.ascontiguousarray(b.reshape(HT, P, 1))
    in_maps = []
    for i in range(NCORES):
        sl = slice(i * NL, (i + 1) * NL)
        in_maps.append(
            {
                "ids": np.ascontiguousarray(ids[sl].reshape(RTOT, P, 1)),
                "pids": np.ascontiguousarray(pids[sl].reshape(RTOT, P, 1)),
                "emb": emb,
                "pemb": pemb,
                "wt": wt,
                "bias": bias,
            }
        )
    return in_maps


def run(inputs, trace=False):
    nc = build_nc()
    in_maps = _prep_inputs(inputs)
    res = run_bass_kernel_spmd(nc, in_maps, list(range(NCORES)), trace=trace)
    out = np.concatenate([res.results[i]["out"] for i in range(NCORES)], axis=0)
    return out, res


def kernel(**inputs):
    out, _ = run(inputs, trace=False)
    return out


# revision 20
# speedup vs baseline: 2.3065x; 1.1341x over previous
"""Trainium2 Bass kernel for nn_AttentiveEncoderPOS (embed+concat+linear+self-attention).

Strategy (8 cores, sequence-parallel with AllGather):
  - Each core gathers/computes only ITS 1024-row slice of
    L = concat(emb[ids], pos[pids]) @ W.T + b, in transposed layout
    (L.T, h on partitions), quantized to fp8 (x32 scale). A per-chunk
    8-core AllGather (fp8 payload) shares all slices while compute runs.
  - Full fp8 L.T stays resident in SBUF in DoubleRow layout [128, 2, N]
    (adjacent h-tile pairs stacked), so score matmuls run fp8 DoubleRow
    (2 contraction rows/cycle). V tiles are transposed out of the same
    resident fp8 L.T on the PE and widened to bf16; exp() output is bf16,
    so attn @ V runs bf16. Phase 2 streams nothing from DRAM.
  - Scores are tiny (|s|<0.025) so exp() without max-subtraction is exact
    softmax; denominator accumulates on the vector engine.
"""

import numpy as np

import concourse.bass as bass
import concourse.mybir as mybir
from concourse import bacc
from concourse.tile import TileContext
from concourse.bass_utils import run_bass_kernel_spmd
from concourse.masks import make_identity

N = 8192
H = 1024
VOCAB = 50257
POS = 64
NCORES = 8
NL = N // NCORES          # 1024 rows (queries) per core
P = 128
HT = H // P               # 8 h tiles
HT2 = HT // 2             # 4 DoubleRow h-pair tiles
K2 = 2 * H
KTI = K2 // P             # 16 contraction tiles for the linear
RTOT = NL // P            # 8 row tiles per core
CHUNK = 512
NCH = NL // CHUNK         # 2 phase-1 chunks
RT = CHUNK // P           # 4 row tiles / chunk
KT = N // P               # 64 key tiles
BLK = 8                   # key tiles per phase-2 block
NBLK = KT // BLK
QTP = NL // P             # 8 q tiles
QH = NL // CHUNK          # 2 score chunks along q
HH = H // CHUNK           # 2 A@V output chunks
FSCALE = 32.0             # fp8 quantization scale for L
KE = 8.0                  # fp8 scale for e' = KE*(exp(s)-1)
SCALE = 1.0 / 32.0        # 1/sqrt(H)
SCALE8 = SCALE / (FSCALE * FSCALE)

BF = mybir.dt.bfloat16
F8 = mybir.dt.float8e4
F32 = mybir.dt.float32
I32 = mybir.dt.int32
EXP = mybir.ActivationFunctionType.Exp
COPY = mybir.ActivationFunctionType.Copy
DR = mybir.MatmulPerfMode.DoubleRow


def build_nc():
    nc = bacc.Bacc()
    ids = nc.declare_dram_parameter("ids", [RTOT, P, 1], I32, isOutput=False)
    pids = nc.declare_dram_parameter("pids", [RTOT, P, 1], I32, isOutput=False)
    emb = nc.declare_dram_parameter("emb", [VOCAB, H], F32, isOutput=False)
    pemb = nc.declare_dram_parameter("pemb", [POS, H], F32, isOutput=False)
    wt = nc.declare_dram_parameter("wt", [K2, H], F32, isOutput=False)  # W.T
    bias = nc.declare_dram_parameter("bias", [HT, P, 1], F32, isOutput=False)
    out = nc.declare_dram_parameter("out", [NL, H], F32, isOutput=True)

    # AllGather bounce buffers (fp8), one pair per 512-row phase-1 chunk so
    # the gather of chunk 0 overlaps phase-1 compute of chunk 1 and phase 2.
    ag_in = [nc.dram_tensor(f"ag_in{c}", [HT, P, CHUNK], F8) for c in range(NCH)]
    ag_out = [
        nc.dram_tensor(
            f"ag_out{c}", [NCORES, HT, P, CHUNK], F8, addr_space="Shared"
        )
        for c in range(NCH)
    ]
    # AllReduce for the global column-sum of 32*L (the attn@V fp8 correction)
    ar_in = nc.dram_tensor("ar_in", [P, HT], F32)
    ar_out = nc.dram_tensor("ar_out", [P, HT], F32, addr_space="Shared")

    with TileContext(nc) as tc:
        with (
            tc.tile_pool(name="const", bufs=1) as const,
            tc.tile_pool(name="qres", bufs=1) as qresp,
        ):
            ident8 = const.tile([P, P], F8)
            make_identity(nc, ident8[:])
            identb = const.tile([P, P], BF)
            make_identity(nc, identb[:])
            ident32 = const.tile([P, P], F32)
            make_identity(nc, ident32[:])
            ones32 = const.tile([P, 1], F32)
            nc.gpsimd.memset(ones32[:], 1.0)
            kerow32 = const.tile([1, P], F32)
            nc.gpsimd.memset(kerow32[:], KE)
            one1 = const.tile([1, 1], F32)
            nc.gpsimd.memset(one1[:], 1.0 / (FSCALE * KE))
            b_sb = const.tile([P, HT], F32)
            nc.sync.dma_start(
                out=b_sb[:].rearrange("p (h u) -> p h u", h=HT),
                in_=bias.rearrange("h p u -> p h u"),
            )
            b32_sb = const.tile([P, HT], F32)
            nc.vector.tensor_scalar_mul(out=b32_sb[:], in0=b_sb[:], scalar1=FSCALE)
            # own fp8 L.T chunk in DoubleRow layout (these are the queries)
            q8 = [
                qresp.tile([P, 2, NL], F8, tag=f"q{h2}", name=f"q{h2}")
                for h2 in range(HT2)
            ]

            # ---------------- Phase 1: own L.T chunk ----------------
            with (
                tc.tile_pool(name="wld", bufs=2) as wld,
                tc.tile_pool(name="wtp", bufs=1) as wtp,
                tc.tile_pool(name="idp", bufs=8) as idp,
                tc.tile_pool(name="xfp", bufs=RTOT + 1) as xfp,
                tc.tile_pool(name="xbp", bufs=RT + 1) as xbp,
                tc.tile_pool(name="xtp", bufs=KTI + 2) as xtp,
                tc.tile_pool(name="tps", bufs=2, space="PSUM") as tps,
                tc.tile_pool(name="mps", bufs=2, space="PSUM") as mps,
            ):
                # ids first, then chunk-0 gathers, so the W loads (on the
                # scalar queue) don't gate the first AllGather.
                idts, pidts = [], []
                for t in range(RTOT):
                    idt = idp.tile([P, 1], I32, tag="id")
                    nc.sync.dma_start(out=idt[:], in_=ids[t])
                    pidt = idp.tile([P, 1], I32, tag="pid")
                    nc.sync.dma_start(out=pidt[:], in_=pids[t])
                    idts.append(idt)
                    pidts.append(pidt)

                # gathers for all row tiles issue before the W loads so the
                # first AllGather isn't gated on 8MB of weight traffic
                xfs = []
                for t in range(RTOT):
                    xf = xfp.tile([P, K2], F32, tag="xf")
                    nc.gpsimd.indirect_dma_start(
                        out=xf[:, 0:H],
                        out_offset=None,
                        in_=emb[:],
                        in_offset=bass.IndirectOffsetOnAxis(
                            ap=idts[t][:, :1], axis=0
                        ),
                    )
                    nc.gpsimd.indirect_dma_start(
                        out=xf[:, H:K2],
                        out_offset=None,
                        in_=pemb[:],
                        in_offset=bass.IndirectOffsetOnAxis(
                            ap=pidts[t][:, :1], axis=0
                        ),
                    )
                    xfs.append(xf)

                wtb = []
                for k in range(KTI):
                    wf = wld.tile([P, H], F32, tag="wld")
                    nc.scalar.dma_start(out=wf[:], in_=wt[k * P : (k + 1) * P, :])
                    wb = wtp.tile([P, H], BF, tag=f"wtb{k}", name=f"wtb{k}")
                    nc.vector.tensor_copy(out=wb[:], in_=wf[:])
                    wtb.append(wb)

                for ch in range(NCH):
                    xbs = []
                    for rt in range(RT):
                        t = ch * RT + rt
                        xb = xbp.tile([P, K2], BF, tag="xb")
                        nc.vector.tensor_copy(out=xb[:], in_=xfs[t][:])
                        xbs.append(xb)
                    xts = []
                    for k in range(KTI):
                        pt = tps.tile([P, CHUNK], BF, tag="tp")
                        for rt in range(RT):
                            nc.tensor.transpose(
                                pt[:, rt * P : (rt + 1) * P],
                                xbs[rt][:, k * P : (k + 1) * P],
                                identb[:],
                            )
                        xt = xtp.tile([P, CHUNK], BF, tag="xt")
                        nc.vector.tensor_copy(out=xt[:], in_=pt[:])
                        xts.append(xt)

                    # linear: L.T[ht, chunk] = sum_k W.T[k,ht].T @ X.T[k,chunk]
                    csl = slice(ch * CHUNK, (ch + 1) * CHUNK)
                    for ht in range(HT):
                        pm = mps.tile([P, CHUNK], F32, tag="mp")
                        for k in range(KTI):
                            nc.tensor.matmul(
                                pm[:],
                                lhsT=wtb[k][:, ht * P : (ht + 1) * P],
                                rhs=xts[k][:],
                                start=(k == 0),
                                stop=(k == KTI - 1),
                            )
                        # fp8 quantize: q8 = 32*(pm + b)
                        nc.vector.tensor_scalar(
                            out=q8[ht // 2][:, ht % 2, csl],
                            in0=pm[:],
                            scalar1=b_sb[:, ht : ht + 1],
                            scalar2=FSCALE,
                            op0=mybir.AluOpType.add,
                            op1=mybir.AluOpType.mult,
                        )
                        nc.sync.dma_start(
                            out=ag_in[ch][ht], in_=q8[ht // 2][:, ht % 2, csl]
                        )
                    # AllGather this 512-row chunk while the next computes
                    nc.gpsimd.collective_compute(
                        "AllGather",
                        mybir.AluOpType.bypass,
                        replica_groups=[list(range(NCORES))],
                        ins=[ag_in[ch][:].opt()],
                        outs=[ag_out[ch][:].opt()],
                    )

                # own-chunk column sums of 32*L (for the attn@V correction):
                # sum q8 over keys on the scalar engine, AllReduce across cores
                vs_own = xtp.tile([P, HT], F32, tag="vso", bufs=1)
                for ht in range(HT):
                    scr8 = xbp.tile([P, NL], F8, tag="scr8", bufs=2)
                    nc.scalar.activation(
                        out=scr8[:],
                        in_=q8[ht // 2][:, ht % 2, :],
                        func=COPY,
                        accum_out=vs_own[:, ht : ht + 1],
                    )
                nc.sync.dma_start(out=ar_in[:], in_=vs_own[:])
                nc.gpsimd.collective_compute(
                    "AllReduce",
                    mybir.AluOpType.add,
                    replica_groups=[list(range(NCORES))],
                    ins=[ar_in[:].opt()],
                    outs=[ar_out[:].opt()],
                )

            # ---------------- Phase 2: attention ----------------
            with (
                tc.tile_pool(name="ltr", bufs=1) as ltr,
                tc.tile_pool(name="ep", bufs=4) as epool,
                tc.tile_pool(name="e8p", bufs=BLK // 2 + 2) as e8p,
                tc.tile_pool(name="vp", bufs=BLK // 2 + 2) as vp,
                tc.tile_pool(name="osb", bufs=QTP) as osb,
                tc.tile_pool(name="fin", bufs=2) as fin,
                tc.tile_pool(name="sps", bufs=3, space="PSUM") as sps,
                tc.tile_pool(name="ops", bufs=2, space="PSUM") as ops,
                tc.tile_pool(name="tvs", bufs=1, space="PSUM") as tvs,
            ):
                # Full fp8 L.T resident in DoubleRow layout: 4 tiles
                # [128, 2, 8192] (8MB), loaded per AllGather chunk.
                lt8 = [
                    ltr.tile([P, 2, N], F8, tag=f"lt{h2}", name=f"lt{h2}")
                    for h2 in range(HT2)
                ]
                for ch in range(NCH):
                    for ht in range(HT):
                        dst = lt8[ht // 2][:, ht % 2, :].rearrange(
                            "p (c g n) -> p c g n", c=NCORES, g=NCH
                        )[:, :, ch, :]
                        nc.sync.dma_start(
                            out=dst,
                            in_=ag_out[ch][:, ht].rearrange("c p n -> p c n"),
                        )

                # key-tile blocks ordered chunk-half first, so the first half
                # only depends on AllGather 0
                blocks = [
                    [c8 * RTOT + half * RT + jj for c8 in (2 * bc, 2 * bc + 1)
                     for jj in range(RT)]
                    for half in range(NCH)
                    for bc in range(NCORES // 2)
                ]

                # global column-sum row of 32*L: vrow32[0, h] = sum_k 32*L[k, h]
                vs_all = fin.tile([P, HT], F32, tag="vsa")
                nc.sync.dma_start(out=vs_all[:], in_=ar_out[:])
                vrow32 = fin.tile([1, H], F32, tag="vrow")
                for ht in range(HT):
                    vtp = tvs.tile([1, P], F32, tag="tv")
                    nc.tensor.transpose(
                        vtp[:], vs_all[:, ht : ht + 1], ident32[:]
                    )
                    nc.vector.tensor_copy(
                        out=vrow32[0:1, ht * P : (ht + 1) * P], in_=vtp[:]
                    )

                csum = fin.tile([P, NL], F32, tag="csum")
                out_sb = [
                    osb.tile([P, H], F32, tag="o", name="o") for _ in range(QTP)
                ]
                for blk, kts in enumerate(blocks):
                    e8s = []
                    v8s = []
                    for j, kt in enumerate(kts):
                        ksl = slice(kt * P, (kt + 1) * P)
                        if j % 2 == 0:
                            e8 = e8p.tile([P, 2, NL], F8, tag="e8")
                            v8 = vp.tile([P, 2, H], F8, tag="v8")
                            e8s.append(e8)
                            v8s.append(v8)
                        eb = epool.tile([P, NL], BF, tag="e")
                        for qh in range(QH):
                            qsl = slice(qh * CHUNK, (qh + 1) * CHUNK)
                            ps = sps.tile([P, CHUNK], F32, tag="sp")
                            for h2 in range(HT2):
                                nc.tensor.matmul(
                                    ps[:],
                                    lhsT=lt8[h2][:, :, ksl],
                                    rhs=q8[h2][:, :, qsl],
                                    start=(h2 == 0),
                                    stop=(h2 == HT2 - 1),
                                    perf_mode=DR,
                                )
                            nc.scalar.activation(
                                out=eb[:, qsl], in_=ps[:], func=EXP, scale=SCALE8
                            )
                        # denominator partial (cross-partition sum at end)
                        if blk == 0 and kts[0] == kt:
                            nc.vector.tensor_copy(out=csum[:], in_=eb[:])
                        else:
                            nc.vector.tensor_add(out=csum[:], in0=csum[:], in1=eb[:])
                        # e' = KE*(exp(s)-1) in fp8 keeps the softmax signal
                        nc.vector.tensor_scalar(
                            out=e8[:, j % 2, :],
                            in0=eb[:],
                            scalar1=-1.0,
                            scalar2=KE,
                            op0=mybir.AluOpType.add,
                            op1=mybir.AluOpType.mult,
                        )
                        # V tile: transpose resident fp8 L.T block to [keys, h].
                        # fp8 transpose requires output element step 2, so the
                        # PSUM tile carries a dummy trailing dim.
                        pt = tvs.tile([P, H, 2], F8, tag="tv")
                        for ht in range(HT):
                            nc.tensor.transpose(
                                pt[:, ht * P : (ht + 1) * P, 0],
                                lt8[ht // 2][:, ht % 2, ksl],
                                ident8[:],
                            )
                        nc.scalar.activation(
                            out=v8[:, j % 2, :], in_=pt[:, :, 0], func=COPY
                        )
                    last = blk == NBLK - 1
                    for qt in range(QTP):
                        po = ops.tile([P, H], F32, tag="op")
                        for j2 in range(BLK // 2):
                            for hh in range(HH):
                                nc.tensor.matmul(
                                    po[:, hh * CHUNK : (hh + 1) * CHUNK],
                                    lhsT=e8s[j2][:, :, qt * P : (qt + 1) * P],
                                    rhs=v8s[j2][:, :, hh * CHUNK : (hh + 1) * CHUNK],
                                    start=(j2 == 0),
                                    stop=(j2 == BLK // 2 - 1 and not last),
                                    perf_mode=DR,
                                )
                        if last:
                            # exact rank-1 term: out_unnorm*32*KE needs
                            # + KE * sum_k 32*L[k, :] added once per chain
                            for hh in range(HH):
                                nc.tensor.matmul(
                                    po[:, hh * CHUNK : (hh + 1) * CHUNK],
                                    lhsT=kerow32[0:1, :],
                                    rhs=vrow32[0:1, hh * CHUNK : (hh + 1) * CHUNK],
                                    start=False,
                                    stop=True,
                                )
                        if blk == 0:
                            nc.vector.tensor_copy(out=out_sb[qt][:], in_=po[:])
                        else:
                            nc.vector.tensor_add(
                                out=out_sb[qt][:], in0=out_sb[qt][:], in1=po[:]
                            )
                # normalize: colsum -> reciprocal row, move q to partitions.
                # V carried x32 and e' x KE (one1 = 1/(32*KE) folds both back).
                rec_row = fin.tile([1, NL], F32, tag="rr")
                for qh in range(QH):
                    qsl = slice(qh * CHUNK, (qh + 1) * CHUNK)
                    psc = sps.tile([1, CHUNK], F32, tag="sp")
                    nc.tensor.matmul(
                        psc[:], lhsT=ones32[:], rhs=csum[:, qsl],
                        start=True, stop=True,
                    )
                    nc.vector.reciprocal(rec_row[0:1, qsl], psc[:])
                for qt in range(QTP):
                    ct = tvs.tile([P, 1], F32, tag="tv")
                    nc.tensor.matmul(
                        ct[:],
                        lhsT=rec_row[0:1, qt * P : (qt + 1) * P],
                        rhs=one1[0:1, 0:1],
                        start=True,
                        stop=True,
                    )
                    rec = fin.tile([P, 1], F32, tag="rec")
                    nc.vector.tensor_copy(out=rec[:], in_=ct[:])
                    nc.vector.tensor_mul(
                        out=out_sb[qt][:],
                        in0=out_sb[qt][:],
                        in1=rec[:, 0:1].to_broadcast([P, H]),
                    )
                    nc.sync.dma_start(
                        out=out[qt * P : (qt + 1) * P, :], in_=out_sb[qt][:]
                    )
    nc.finalize()
    return nc


def _prep_inputs(inputs):
    ids = np.asarray(inputs["input_ids"]).astype(np.int32)
    pids = np.asarray(inputs["pos_ids"]).astype(np.int32)
    emb = np.asarray(inputs["emb"], dtype=np.float32)
    pemb = np.asarray(inputs["pos_emb"], dtype=np.float32)
    W = np.asarray(inputs["W"], dtype=np.float32)
    b = np.asarray(inputs["b"], dtype=np.float32)
    wt = np.ascontiguousarray(W.T)                      # [2H, H]
    bias = np.ascontiguousarray(b.reshape(HT, P, 1))
    in_maps = []
    for i in range(NCORES):
        sl = slice(i * NL, (i + 1) * NL)
        in_maps.append(
            {
                "ids": np.ascontiguousarray(ids[sl].reshape(RTOT, P, 1)),
                "pids": np.ascontiguousarray(pids[sl].reshape(RTOT, P, 1)),
                "emb": emb,
                "pemb": pemb,
                "wt": wt,
                "bias": bias,
            }
        )
    return in_maps


def run(inputs, trace=False):
    nc = build_nc()
    in_maps = _prep_inputs(inputs)
    res = run_bass_kernel_spmd(nc, in_maps, list(range(NCORES)), trace=trace)
    out = np.concatenate([res.results[i]["out"] for i in range(NCORES)], axis=0)
    return out, res


def kernel(**inputs):
    out, _ = run(inputs, trace=False)
    return out


# revision 28
# speedup vs baseline: 2.3924x; 1.0372x over previous
"""Trainium2 Bass kernel for nn_AttentiveEncoderPOS (embed+concat+linear+self-attention).

Strategy (8 cores, sequence-parallel with AllGather):
  - Each core gathers/computes only ITS 1024-row slice of
    L = concat(emb[ids], pos[pids]) @ W.T + b, in transposed layout
    (L.T, h on partitions), quantized to fp8 (x32 scale). A per-chunk
    8-core AllGather (fp8 payload) shares all slices while compute runs.
  - Full fp8 L.T stays resident in SBUF in DoubleRow layout [128, 2, N]
    (adjacent h-tile pairs stacked), so score matmuls run fp8 DoubleRow
    (2 contraction rows/cycle). V tiles are transposed out of the same
    resident fp8 L.T on the PE and widened to bf16; exp() output is bf16,
    so attn @ V runs bf16. Phase 2 streams nothing from DRAM.
  - Scores are tiny (|s|<0.025) so exp() without max-subtraction is exact
    softmax; denominator accumulates on the vector engine.
"""

import numpy as np

import concourse.bass as bass
import concourse.mybir as mybir
from concourse import bacc
from concourse.tile import TileContext
from concourse.bass_utils import run_bass_kernel_spmd
from concourse.masks import make_identity

N = 8192
H = 1024
VOCAB = 50257
POS = 64
NCORES = 8
NL = N // NCORES          # 1024 rows (queries) per core
P = 128
HT = H // P               # 8 h tiles
HT2 = HT // 2             # 4 DoubleRow h-pair tiles
K2 = 2 * H
KTI = K2 // P             # 16 contraction tiles for the linear
RTOT = NL // P            # 8 row tiles per core
CHUNK = 512
NCH = NL // CHUNK         # 2 phase-1 chunks
RT = CHUNK // P           # 4 row tiles / chunk
KT = N // P               # 64 key tiles
BLK = 8                   # key tiles per phase-2 block
NBLK = KT // BLK
QTP = NL // P             # 8 q tiles
QH = NL // CHUNK          # 2 score chunks along q
HH = H // CHUNK           # 2 A@V output chunks
FSCALE = 32.0             # fp8 quantization scale for L
KE = 8.0                  # fp8 scale for e' = KE*(exp(s)-1)
SCALE = 1.0 / 32.0        # 1/sqrt(H)
SCALE8 = SCALE / (FSCALE * FSCALE)

BF = mybir.dt.bfloat16
F8 = mybir.dt.float8e4
F32 = mybir.dt.float32
I32 = mybir.dt.int32
EXP = mybir.ActivationFunctionType.Exp
COPY = mybir.ActivationFunctionType.Copy
DR = mybir.MatmulPerfMode.DoubleRow


def build_nc():
    nc = bacc.Bacc()
    ids = nc.declare_dram_parameter("ids", [RTOT, P, 1], I32, isOutput=False)
    pids = nc.declare_dram_parameter("pids", [RTOT, P, 1], I32, isOutput=False)
    emb = nc.declare_dram_parameter("emb", [VOCAB, H], F32, isOutput=False)
    pemb = nc.declare_dram_parameter("pemb", [POS, H], F32, isOutput=False)
    wt = nc.declare_dram_parameter("wt", [K2, H], F32, isOutput=False)  # W.T
    bias = nc.declare_dram_parameter("bias", [HT, P, 1], F32, isOutput=False)
    out = nc.declare_dram_parameter("out", [NL, H], F32, isOutput=True)

    # AllGather bounce buffers (fp8), one pair per 512-row phase-1 chunk so
    # the gather of chunk 0 overlaps phase-1 compute of chunk 1 and phase 2.
    # Units 0..HT-1 carry L.T tiles; units HT.. carry V-natural row tiles
    # (each [P, H] V tile spans two [P, CHUNK] units).
    AGU = HT + RT * (H // CHUNK)
    ag_in = [nc.dram_tensor(f"ag_in{c}", [AGU, P, CHUNK], F8) for c in range(NCH)]
    ag_out = [
        nc.dram_tensor(
            f"ag_out{c}", [NCORES, AGU, P, CHUNK], F8, addr_space="Shared"
        )
        for c in range(NCH)
    ]
    # AllReduce for the global column-sum of 32*L (the attn@V fp8 correction)
    ar_in = nc.dram_tensor("ar_in", [P, HT], F32)
    ar_out = nc.dram_tensor("ar_out", [P, HT], F32, addr_space="Shared")

    with TileContext(nc) as tc:
        with (
            tc.tile_pool(name="const", bufs=1) as const,
            tc.tile_pool(name="qres", bufs=1) as qresp,
        ):
            ident8 = const.tile([P, P], F8)
            make_identity(nc, ident8[:])
            ident32 = const.tile([P, P], F32)
            make_identity(nc, ident32[:])
            ones32 = const.tile([P, 1], F32)
            nc.gpsimd.memset(ones32[:], 1.0)
            kerow32 = const.tile([1, P], F32)
            nc.gpsimd.memset(kerow32[:], KE)
            one1 = const.tile([1, 1], F32)
            nc.gpsimd.memset(one1[:], 1.0 / (FSCALE * KE))
            b_sb = const.tile([P, HT], F32)
            nc.sync.dma_start(
                out=b_sb[:].rearrange("p (h u) -> p h u", h=HT),
                in_=bias.rearrange("h p u -> p h u"),
            )
            # the fp8 linear computes 1024*(X@W.T); fold bias pre-scaled
            b1024_sb = const.tile([P, HT], F32)
            nc.vector.tensor_scalar_mul(
                out=b1024_sb[:], in0=b_sb[:], scalar1=FSCALE * FSCALE
            )
            # own fp8 L.T chunk in DoubleRow layout (these are the queries)
            q8 = [
                qresp.tile([P, 2, NL], F8, tag=f"q{h2}", name=f"q{h2}")
                for h2 in range(HT2)
            ]

            # ---------------- Phase 1: own L.T chunk ----------------
            with (
                tc.tile_pool(name="wld", bufs=2) as wld,
                tc.tile_pool(name="wtp", bufs=1) as wtp,
                tc.tile_pool(name="idp", bufs=8) as idp,
                tc.tile_pool(name="xfp", bufs=RTOT + 1) as xfp,
                tc.tile_pool(name="xbp", bufs=RT + 1) as xbp,
                tc.tile_pool(name="xtp", bufs=KTI + 2) as xtp,
                tc.tile_pool(name="tps", bufs=2, space="PSUM") as tps,
                tc.tile_pool(name="mps", bufs=2, space="PSUM") as mps,
            ):
                # ids first, then chunk-0 gathers, so the W loads (on the
                # scalar queue) don't gate the first AllGather.
                idts, pidts = [], []
                for t in range(RTOT):
                    idt = idp.tile([P, 1], I32, tag="id")
                    nc.sync.dma_start(out=idt[:], in_=ids[t])
                    pidt = idp.tile([P, 1], I32, tag="pid")
                    nc.sync.dma_start(out=pidt[:], in_=pids[t])
                    idts.append(idt)
                    pidts.append(pidt)

                # gathers for all row tiles issue before the W loads so the
                # first AllGather isn't gated on 8MB of weight traffic
                xfs = []
                for t in range(RTOT):
                    xf = xfp.tile([P, K2], F32, tag="xf")
                    nc.gpsimd.indirect_dma_start(
                        out=xf[:, 0:H],
                        out_offset=None,
                        in_=emb[:],
                        in_offset=bass.IndirectOffsetOnAxis(
                            ap=idts[t][:, :1], axis=0
                        ),
                    )
                    nc.gpsimd.indirect_dma_start(
                        out=xf[:, H:K2],
                        out_offset=None,
                        in_=pemb[:],
                        in_offset=bass.IndirectOffsetOnAxis(
                            ap=pidts[t][:, :1], axis=0
                        ),
                    )
                    xfs.append(xf)

                # W.T in fp8 DoubleRow layout (x32), via f32 staging
                w8 = []
                for k in range(KTI):
                    wf = wld.tile([P, H], F32, tag="wld")
                    nc.scalar.dma_start(out=wf[:], in_=wt[k * P : (k + 1) * P, :])
                    if k % 2 == 0:
                        w8.append(
                            wtp.tile([P, 2, H], F8, tag=f"w8_{k//2}", name=f"w8_{k//2}")
                        )
                    nc.vector.tensor_scalar_mul(
                        out=w8[k // 2][:, k % 2, :], in0=wf[:], scalar1=FSCALE
                    )

                for ch in range(NCH):
                    x8bs = []
                    for rt in range(RT):
                        t = ch * RT + rt
                        x8b = xbp.tile([P, K2], F8, tag="xb")
                        nc.vector.tensor_scalar_mul(
                            out=x8b[:], in0=xfs[t][:], scalar1=FSCALE
                        )
                        x8bs.append(x8b)
                    x8ts = []
                    for k2 in range(KTI // 2):
                        x8t = xtp.tile([P, 2, CHUNK], F8, tag="xt")
                        for r in range(2):
                            pt = tps.tile([P, CHUNK, 2], F8, tag="tp")
                            k = 2 * k2 + r
                            for rt in range(RT):
                                nc.tensor.transpose(
                                    pt[:, rt * P : (rt + 1) * P, 0],
                                    x8bs[rt][:, k * P : (k + 1) * P],
                                    ident8[:],
                                )
                            nc.scalar.activation(
                                out=x8t[:, r, :], in_=pt[:, :, 0], func=COPY
                            )
                        x8ts.append(x8t)

                    # linear (fp8 DR): 1024*L.T[ht, chunk]
                    csl = slice(ch * CHUNK, (ch + 1) * CHUNK)
                    for ht in range(HT):
                        pm = mps.tile([P, CHUNK], F32, tag="mp")
                        for k2 in range(KTI // 2):
                            nc.tensor.matmul(
                                pm[:],
                                lhsT=w8[k2][:, :, ht * P : (ht + 1) * P],
                                rhs=x8ts[k2][:],
                                start=(k2 == 0),
                                stop=(k2 == KTI // 2 - 1),
                                perf_mode=DR,
                            )
                        # fp8 quantize: q8 = 32*(pm/1024 + b) = (pm + 1024b)/32
                        nc.vector.tensor_scalar(
                            out=q8[ht // 2][:, ht % 2, csl],
                            in0=pm[:],
                            scalar1=b1024_sb[:, ht : ht + 1],
                            scalar2=1.0 / FSCALE,
                            op0=mybir.AluOpType.add,
                            op1=mybir.AluOpType.mult,
                        )
                        nc.sync.dma_start(
                            out=ag_in[ch][ht], in_=q8[ht // 2][:, ht % 2, csl]
                        )
                    # V-natural tiles for this chunk (so phase 2 needn't
                    # transpose): transpose own L.T rows back to [keys, h]
                    for rt in range(RT):
                        rsl = slice(ch * CHUNK + rt * P, ch * CHUNK + (rt + 1) * P)
                        ptv = tps.tile([P, H, 2], F8, tag="tpv")
                        for ht in range(HT):
                            nc.tensor.transpose(
                                ptv[:, ht * P : (ht + 1) * P, 0],
                                q8[ht // 2][:, ht % 2, rsl],
                                ident8[:],
                            )
                        vn = xbp.tile([P, H], F8, tag="vn", bufs=3)
                        nc.scalar.activation(
                            out=vn[:], in_=ptv[:, :, 0], func=COPY
                        )
                        for u in range(2):
                            nc.sync.dma_start(
                                out=ag_in[ch][HT + 2 * rt + u],
                                in_=vn[:, u * CHUNK : (u + 1) * CHUNK],
                            )
                    # AllGather this 512-row chunk while the next computes
                    nc.gpsimd.collective_compute(
                        "AllGather",
                        mybir.AluOpType.bypass,
                        replica_groups=[list(range(NCORES))],
                        ins=[ag_in[ch][:].opt()],
                        outs=[ag_out[ch][:].opt()],
                    )

                # own-chunk column sums of 32*L (for the attn@V correction):
                # sum q8 over keys on the scalar engine, AllReduce across cores
                vs_own = xtp.tile([P, HT], F32, tag="vso", bufs=1)
                for ht in range(HT):
                    scr8 = xbp.tile([P, NL], F8, tag="scr8", bufs=2)
                    nc.scalar.activation(
                        out=scr8[:],
                        in_=q8[ht // 2][:, ht % 2, :],
                        func=COPY,
                        accum_out=vs_own[:, ht : ht + 1],
                    )
                nc.sync.dma_start(out=ar_in[:], in_=vs_own[:])
                nc.gpsimd.collective_compute(
                    "AllReduce",
                    mybir.AluOpType.add,
                    replica_groups=[list(range(NCORES))],
                    ins=[ar_in[:].opt()],
                    outs=[ar_out[:].opt()],
                )

            # ---------------- Phase 2: attention ----------------
            with (
                tc.tile_pool(name="ltr", bufs=1) as ltr,
                tc.tile_pool(name="ep", bufs=4) as epool,
                tc.tile_pool(name="e8p", bufs=BLK // 2 + 2) as e8p,
                tc.tile_pool(name="vp", bufs=BLK // 2 + 2) as vp,
                tc.tile_pool(name="osb", bufs=QTP) as osb,
                tc.tile_pool(name="fin", bufs=2) as fin,
                tc.tile_pool(name="sps", bufs=3, space="PSUM") as sps,
                tc.tile_pool(name="ops", bufs=2, space="PSUM") as ops,
                tc.tile_pool(name="tvs", bufs=1, space="PSUM") as tvs,
            ):
                # Full fp8 L.T resident in DoubleRow layout: 4 tiles
                # [128, 2, 8192] (8MB), loaded per AllGather chunk.
                lt8 = [
                    ltr.tile([P, 2, N], F8, tag=f"lt{h2}", name=f"lt{h2}")
                    for h2 in range(HT2)
                ]
                for ch in range(NCH):
                    for ht in range(HT):
                        dst = lt8[ht // 2][:, ht % 2, :].rearrange(
                            "p (c g n) -> p c g n", c=NCORES, g=NCH
                        )[:, :, ch, :]
                        nc.sync.dma_start(
                            out=dst,
                            in_=ag_out[ch][:, ht].rearrange("c p n -> p c n"),
                        )

                # key-tile blocks ordered chunk-half first, so the first half
                # only depends on AllGather 0
                blocks = [
                    [c8 * RTOT + half * RT + jj for c8 in (2 * bc, 2 * bc + 1)
                     for jj in range(RT)]
                    for half in range(NCH)
                    for bc in range(NCORES // 2)
                ]

                # global column-sum row of 32*L: vrow32[0, h] = sum_k 32*L[k, h]
                vs_all = fin.tile([P, HT], F32, tag="vsa")
                nc.sync.dma_start(out=vs_all[:], in_=ar_out[:])
                vrow32 = fin.tile([1, H], F32, tag="vrow")
                for ht in range(HT):
                    vtp = tvs.tile([1, P], F32, tag="tv")
                    nc.tensor.transpose(
                        vtp[:], vs_all[:, ht : ht + 1], ident32[:]
                    )
                    nc.vector.tensor_copy(
                        out=vrow32[0:1, ht * P : (ht + 1) * P], in_=vtp[:]
                    )

                csum = fin.tile([P, NL], F32, tag="csum")
                out_sb = [
                    osb.tile([P, H], F32, tag="o", name="o") for _ in range(QTP)
                ]
                for blk, kts in enumerate(blocks):
                    e8s = []
                    v8s = []
                    for j, kt in enumerate(kts):
                        ksl = slice(kt * P, (kt + 1) * P)
                        if j % 2 == 0:
                            e8 = e8p.tile([P, 2, NL], F8, tag="e8")
                            v8 = vp.tile([P, 2, H], F8, tag="v8")
                            e8s.append(e8)
                            v8s.append(v8)
                        eb = epool.tile([P, NL], BF, tag="e")
                        for qh in range(QH):
                            qsl = slice(qh * CHUNK, (qh + 1) * CHUNK)
                            ps = sps.tile([P, CHUNK], F32, tag="sp")
                            for h2 in range(HT2):
                                nc.tensor.matmul(
                                    ps[:],
                                    lhsT=lt8[h2][:, :, ksl],
                                    rhs=q8[h2][:, :, qsl],
                                    start=(h2 == 0),
                                    stop=(h2 == HT2 - 1),
                                    perf_mode=DR,
                                )
                            nc.scalar.activation(
                                out=eb[:, qsl], in_=ps[:], func=EXP, scale=SCALE8
                            )
                        # denominator partial (cross-partition sum at end)
                        if blk == 0 and kts[0] == kt:
                            nc.vector.tensor_copy(out=csum[:], in_=eb[:])
                        else:
                            nc.vector.tensor_add(out=csum[:], in0=csum[:], in1=eb[:])
                        # e' = KE*(exp(s)-1) in fp8 keeps the softmax signal
                        nc.vector.tensor_scalar(
                            out=e8[:, j % 2, :],
                            in0=eb[:],
                            scalar1=-1.0,
                            scalar2=KE,
                            op0=mybir.AluOpType.add,
                            op1=mybir.AluOpType.mult,
                        )
                        # V tile: direct load of the AllGathered V-natural rows
                        c, rem = kt // RTOT, kt % RTOT
                        g, jj = rem // RT, rem % RT
                        for u in range(2):
                            nc.gpsimd.dma_start(
                                out=v8[:, j % 2, u * CHUNK : (u + 1) * CHUNK],
                                in_=ag_out[g][c, HT + 2 * jj + u],
                            )
                    last = blk == NBLK - 1
                    for qt in range(QTP):
                        po = ops.tile([P, H], F32, tag="op")
                        for j2 in range(BLK // 2):
                            for hh in range(HH):
                                nc.tensor.matmul(
                                    po[:, hh * CHUNK : (hh + 1) * CHUNK],
                                    lhsT=e8s[j2][:, :, qt * P : (qt + 1) * P],
                                    rhs=v8s[j2][:, :, hh * CHUNK : (hh + 1) * CHUNK],
                                    start=(j2 == 0),
                                    stop=(j2 == BLK // 2 - 1 and not last),
                                    perf_mode=DR,
                                )
                        if last:
                            # exact rank-1 term: out_unnorm*32*KE needs
                            # + KE * sum_k 32*L[k, :] added once per chain
                            for hh in range(HH):
                                nc.tensor.matmul(
                                    po[:, hh * CHUNK : (hh + 1) * CHUNK],
                                    lhsT=kerow32[0:1, :],
                                    rhs=vrow32[0:1, hh * CHUNK : (hh + 1) * CHUNK],
                                    start=False,
                                    stop=True,
                                )
                        if blk == 0:
                            nc.vector.tensor_copy(out=out_sb[qt][:], in_=po[:])
                        else:
                            nc.vector.tensor_add(
                                out=out_sb[qt][:], in0=out_sb[qt][:], in1=po[:]
                            )
                # normalize: colsum -> reciprocal row, move q to partitions.
                # V carried x32 and e' x KE (one1 = 1/(32*KE) folds both back).
                rec_row = fin.tile([1, NL], F32, tag="rr")
                for qh in range(QH):
                    qsl = slice(qh * CHUNK, (qh + 1) * CHUNK)
                    psc = sps.tile([1, CHUNK], F32, tag="sp")
                    nc.tensor.matmul(
                        psc[:], lhsT=ones32[:], rhs=csum[:, qsl],
                        start=True, stop=True,
                    )
                    nc.vector.reciprocal(rec_row[0:1, qsl], psc[:])
                for qt in range(QTP):
                    ct = tvs.tile([P, 1], F32, tag="tv")
                    nc.tensor.matmul(
                        ct[:],
                        lhsT=rec_row[0:1, qt * P : (qt + 1) * P],
                        rhs=one1[0:1, 0:1],
                        start=True,
                        stop=True,
                    )
                    rec = fin.tile([P, 1], F32, tag="rec")
                    nc.vector.tensor_copy(out=rec[:], in_=ct[:])
                    nc.vector.tensor_mul(
                        out=out_sb[qt][:],
                        in0=out_sb[qt][:],
                        in1=rec[:, 0:1].to_broadcast([P, H]),
                    )
                    nc.sync.dma_start(
                        out=out[qt * P : (qt + 1) * P, :], in_=out_sb[qt][:]
                    )
    nc.finalize()
    return nc


def _prep_inputs(inputs):
    ids = np.asarray(inputs["input_ids"]).astype(np.int32)
    pids = np.asarray(inputs["pos_ids"]).astype(np.int32)
    emb = np.asarray(inputs["emb"], dtype=np.float32)
    pemb = np.asarray(inputs["pos_emb"], dtype=np.float32)
    W = np.asarray(inputs["W"], dtype=np.float32)
    b = np.asarray(inputs["b"], dtype=np.float32)
    wt = np.ascontiguousarray(W.T)                      # [2H, H]
    bias = np.ascontiguousarray(b.reshape(HT, P, 1))
    in_maps = []
    for i in range(NCORES):
        sl = slice(i * NL, (i + 1) * NL)
        in_maps.append(
            {
                "ids": np.ascontiguousarray(ids[sl].reshape(RTOT, P, 1)),
                "pids": np.ascontiguousarray(pids[sl].reshape(RTOT, P, 1)),
                "emb": emb,
                "pemb": pemb,
                "wt": wt,
                "bias": bias,
            }
        )
    return in_maps


def run(inputs, trace=False):
    nc = build_nc()
    in_maps = _prep_inputs(inputs)
    res = run_bass_kernel_spmd(nc, in_maps, list(range(NCORES)), trace=trace)
    out = np.concatenate([res.results[i]["out"] for i in range(NCORES)], axis=0)
    return out, res


def kernel(**inputs):
    out, _ = run(inputs, trace=False)
    return out


# revision 35
# speedup vs baseline: 2.4871x; 1.0396x over previous
"""Trainium2 Bass kernel for nn_AttentiveEncoderPOS (embed+concat+linear+self-attention).

Strategy (8 cores, sequence-parallel with AllGather):
  - Each core gathers/computes only ITS 1024-row slice of
    L = concat(emb[ids], pos[pids]) @ W.T + b, in transposed layout
    (L.T, h on partitions), quantized to fp8 (x32 scale). A per-chunk
    8-core AllGather (fp8 payload) shares all slices while compute runs.
  - Full fp8 L.T stays resident in SBUF in DoubleRow layout [128, 2, N]
    (adjacent h-tile pairs stacked), so score matmuls run fp8 DoubleRow
    (2 contraction rows/cycle). V tiles are transposed out of the same
    resident fp8 L.T on the PE and widened to bf16; exp() output is bf16,
    so attn @ V runs bf16. Phase 2 streams nothing from DRAM.
  - Scores are tiny (|s|<0.025) so exp() without max-subtraction is exact
    softmax; denominator accumulates on the vector engine.
"""

import numpy as np

import concourse.bass as bass
import concourse.mybir as mybir
from concourse import bacc
from concourse.tile import TileContext
from concourse.bass_utils import run_bass_kernel_spmd
from concourse.masks import make_identity

N = 8192
H = 1024
VOCAB = 50257
POS = 64
NCORES = 8
NL = N // NCORES          # 1024 rows (queries) per core
P = 128
HT = H // P               # 8 h tiles
HT2 = HT // 2             # 4 DoubleRow h-pair tiles
K2 = 2 * H
KTI = K2 // P             # 16 contraction tiles for the linear
RTOT = NL // P            # 8 row tiles per core
CHUNK = 512
NCH = NL // CHUNK         # 2 phase-1 chunks
RT = CHUNK // P           # 4 row tiles / chunk
KT = N // P               # 64 key tiles
BLK = 8                   # key tiles per phase-2 block
NBLK = KT // BLK
QTP = NL // P             # 8 q tiles
QH = NL // CHUNK          # 2 score chunks along q
HH = H // CHUNK           # 2 A@V output chunks
FSCALE = 32.0             # fp8 quantization scale for L
KE = 8.0                  # fp8 scale for e' = KE*(exp(s)-1)
SCALE = 1.0 / 32.0        # 1/sqrt(H)
SCALE8 = SCALE / (FSCALE * FSCALE)

BF = mybir.dt.bfloat16
F8 = mybir.dt.float8e4
F32 = mybir.dt.float32
I32 = mybir.dt.int32
EXP = mybir.ActivationFunctionType.Exp
COPY = mybir.ActivationFunctionType.Copy
DR = mybir.MatmulPerfMode.DoubleRow


def build_nc():
    nc = bacc.Bacc()
    ids = nc.declare_dram_parameter("ids", [RTOT, P, 1], I32, isOutput=False)
    pids = nc.declare_dram_parameter("pids", [RTOT, P, 1], I32, isOutput=False)
    # emb/pemb/wt arrive pre-quantized to fp8 (x32) from the host
    emb = nc.declare_dram_parameter("emb", [VOCAB, H], F8, isOutput=False)
    pemb = nc.declare_dram_parameter("pemb", [POS, H], F8, isOutput=False)
    wt = nc.declare_dram_parameter("wt", [K2, H], F8, isOutput=False)  # 32*W.T
    bias = nc.declare_dram_parameter("bias", [HT, P, 1], F32, isOutput=False)
    out = nc.declare_dram_parameter("out", [NL, H], F32, isOutput=True)

    # AllGather bounce buffers (fp8), one pair per 512-row phase-1 chunk so
    # the gather of chunk 0 overlaps phase-1 compute of chunk 1 and phase 2.
    # L.T tiles and V-natural row tiles travel in separate collectives so
    # score matmuls only wait on the L gather.
    ag_in = [nc.dram_tensor(f"ag_in{c}", [HT, P, CHUNK], F8) for c in range(NCH)]
    ag_out = [
        nc.dram_tensor(
            f"ag_out{c}", [NCORES, HT, P, CHUNK], F8, addr_space="Shared"
        )
        for c in range(NCH)
    ]
    agv_in = [nc.dram_tensor(f"agv_in{c}", [RT, P, H], F8) for c in range(NCH)]
    agv_out = [
        nc.dram_tensor(
            f"agv_out{c}", [NCORES, RT, P, H], F8, addr_space="Shared"
        )
        for c in range(NCH)
    ]
    # AllReduce for the global column-sum of 32*L (the attn@V fp8 correction)
    ar_in = nc.dram_tensor("ar_in", [P, HT], F32)
    ar_out = nc.dram_tensor("ar_out", [P, HT], F32, addr_space="Shared")

    with TileContext(nc) as tc:
        with (
            tc.tile_pool(name="const", bufs=1) as const,
            tc.tile_pool(name="qres", bufs=1) as qresp,
        ):
            ident8 = const.tile([P, P], F8)
            make_identity(nc, ident8[:])
            ident32 = const.tile([P, P], F32)
            make_identity(nc, ident32[:])
            ones32 = const.tile([P, 1], F32)
            nc.gpsimd.memset(ones32[:], 1.0)
            kerow32 = const.tile([1, P], F32)
            nc.gpsimd.memset(kerow32[:], KE)
            one1 = const.tile([1, 1], F32)
            nc.gpsimd.memset(one1[:], 1.0 / (FSCALE * KE))
            b_sb = const.tile([P, HT], F32)
            nc.sync.dma_start(
                out=b_sb[:].rearrange("p (h u) -> p h u", h=HT),
                in_=bias.rearrange("h p u -> p h u"),
            )
            # the fp8 linear computes 1024*(X@W.T); fold bias pre-scaled
            b1024_sb = const.tile([P, HT], F32)
            nc.vector.tensor_scalar_mul(
                out=b1024_sb[:], in0=b_sb[:], scalar1=FSCALE * FSCALE
            )
            # own fp8 L.T chunk in DoubleRow layout (these are the queries)
            q8 = [
                qresp.tile([P, 2, NL], F8, tag=f"q{h2}", name=f"q{h2}")
                for h2 in range(HT2)
            ]

            # ---------------- Phase 1: own L.T chunk ----------------
            with (
                tc.tile_pool(name="wtp", bufs=1) as wtp,
                tc.tile_pool(name="idp", bufs=8) as idp,
                tc.tile_pool(name="xfp", bufs=RTOT + 1) as xfp,
                tc.tile_pool(name="xbp", bufs=RT + 1) as xbp,
                tc.tile_pool(name="xtp", bufs=KTI + 2) as xtp,
                tc.tile_pool(name="tps", bufs=2, space="PSUM") as tps,
                tc.tile_pool(name="mps", bufs=2, space="PSUM") as mps,
            ):
                # ids first, then chunk-0 gathers, so the W loads (on the
                # scalar queue) don't gate the first AllGather.
                idts, pidts = [], []
                for t in range(RTOT):
                    idt = idp.tile([P, 1], I32, tag="id")
                    nc.sync.dma_start(out=idt[:], in_=ids[t])
                    pidt = idp.tile([P, 1], I32, tag="pid")
                    nc.sync.dma_start(out=pidt[:], in_=pids[t])
                    idts.append(idt)
                    pidts.append(pidt)

                # gathers for all row tiles (fp8 tables) issue first, then W
                xfs = []
                for t in range(RTOT):
                    xf = xfp.tile([P, K2], F8, tag="xf")
                    nc.gpsimd.indirect_dma_start(
                        out=xf[:, 0:H],
                        out_offset=None,
                        in_=emb[:],
                        in_offset=bass.IndirectOffsetOnAxis(
                            ap=idts[t][:, :1], axis=0
                        ),
                    )
                    nc.gpsimd.indirect_dma_start(
                        out=xf[:, H:K2],
                        out_offset=None,
                        in_=pemb[:],
                        in_offset=bass.IndirectOffsetOnAxis(
                            ap=pidts[t][:, :1], axis=0
                        ),
                    )
                    xfs.append(xf)

                # 32*W.T, fp8, straight into DoubleRow layout
                w8 = []
                for k in range(KTI):
                    if k % 2 == 0:
                        w8.append(
                            wtp.tile([P, 2, H], F8, tag=f"w8_{k//2}", name=f"w8_{k//2}")
                        )
                    nc.scalar.dma_start(
                        out=w8[k // 2][:, k % 2, :], in_=wt[k * P : (k + 1) * P, :]
                    )

                for ch in range(NCH):
                    x8bs = [xfs[ch * RT + rt] for rt in range(RT)]
                    x8ts = []
                    for k2 in range(KTI // 2):
                        x8t = xtp.tile([P, 2, CHUNK], F8, tag="xt")
                        for r in range(2):
                            pt = tps.tile([P, CHUNK, 2], F8, tag="tp")
                            k = 2 * k2 + r
                            for rt in range(RT):
                                nc.tensor.transpose(
                                    pt[:, rt * P : (rt + 1) * P, 0],
                                    x8bs[rt][:, k * P : (k + 1) * P],
                                    ident8[:],
                                )
                            nc.scalar.activation(
                                out=x8t[:, r, :], in_=pt[:, :, 0], func=COPY
                            )
                        x8ts.append(x8t)

                    # linear (fp8 DR): 1024*L.T[ht, chunk]
                    csl = slice(ch * CHUNK, (ch + 1) * CHUNK)
                    for ht in range(HT):
                        pm = mps.tile([P, CHUNK], F32, tag="mp")
                        for k2 in range(KTI // 2):
                            nc.tensor.matmul(
                                pm[:],
                                lhsT=w8[k2][:, :, ht * P : (ht + 1) * P],
                                rhs=x8ts[k2][:],
                                start=(k2 == 0),
                                stop=(k2 == KTI // 2 - 1),
                                perf_mode=DR,
                            )
                        # fp8 quantize: q8 = 32*(pm/1024 + b) = (pm + 1024b)/32
                        nc.vector.tensor_scalar(
                            out=q8[ht // 2][:, ht % 2, csl],
                            in0=pm[:],
                            scalar1=b1024_sb[:, ht : ht + 1],
                            scalar2=1.0 / FSCALE,
                            op0=mybir.AluOpType.add,
                            op1=mybir.AluOpType.mult,
                        )
                        nc.sync.dma_start(
                            out=ag_in[ch][ht], in_=q8[ht // 2][:, ht % 2, csl]
                        )
                    # AllGather the L.T chunk first: scores only need this one
                    nc.gpsimd.collective_compute(
                        "AllGather",
                        mybir.AluOpType.bypass,
                        replica_groups=[list(range(NCORES))],
                        ins=[ag_in[ch][:].opt()],
                        outs=[ag_out[ch][:].opt()],
                    )
                    # V-natural tiles for this chunk (so phase 2 needn't
                    # transpose): transpose own L.T rows back to [keys, h]
                    for rt in range(RT):
                        rsl = slice(ch * CHUNK + rt * P, ch * CHUNK + (rt + 1) * P)
                        ptv = tps.tile([P, H, 2], F8, tag="tpv")
                        for ht in range(HT):
                            nc.tensor.transpose(
                                ptv[:, ht * P : (ht + 1) * P, 0],
                                q8[ht // 2][:, ht % 2, rsl],
                                ident8[:],
                            )
                        vn = xbp.tile([P, H], F8, tag="vn", bufs=3)
                        nc.scalar.activation(
                            out=vn[:], in_=ptv[:, :, 0], func=COPY
                        )
                        nc.sync.dma_start(out=agv_in[ch][rt], in_=vn[:])
                    nc.gpsimd.collective_compute(
                        "AllGather",
                        mybir.AluOpType.bypass,
                        replica_groups=[list(range(NCORES))],
                        ins=[agv_in[ch][:].opt()],
                        outs=[agv_out[ch][:].opt()],
                    )

                # own-chunk column sums of 32*L (for the attn@V correction):
                # sum q8 over keys on the scalar engine, AllReduce across cores
                vs_own = xtp.tile([P, HT], F32, tag="vso", bufs=1)
                for ht in range(HT):
                    scr8 = xbp.tile([P, NL], F8, tag="scr8", bufs=2)
                    nc.scalar.activation(
                        out=scr8[:],
                        in_=q8[ht // 2][:, ht % 2, :],
                        func=COPY,
                        accum_out=vs_own[:, ht : ht + 1],
                    )
                nc.sync.dma_start(out=ar_in[:], in_=vs_own[:])
                nc.gpsimd.collective_compute(
                    "AllReduce",
                    mybir.AluOpType.add,
                    replica_groups=[list(range(NCORES))],
                    ins=[ar_in[:].opt()],
                    outs=[ar_out[:].opt()],
                )

            # ---------------- Phase 2: attention ----------------
            with (
                tc.tile_pool(name="ltr", bufs=1) as ltr,
                tc.tile_pool(name="ep", bufs=4) as epool,
                tc.tile_pool(name="e8p", bufs=BLK // 2 + 2) as e8p,
                tc.tile_pool(name="vp", bufs=BLK // 2 + 2) as vp,
                tc.tile_pool(name="osb", bufs=QTP) as osb,
                tc.tile_pool(name="fin", bufs=2) as fin,
                tc.tile_pool(name="sps", bufs=3, space="PSUM") as sps,
                tc.tile_pool(name="ops", bufs=2, space="PSUM") as ops,
                tc.tile_pool(name="tvs", bufs=1, space="PSUM") as tvs,
            ):
                # Full fp8 L.T resident in DoubleRow layout: 4 tiles
                # [128, 2, 8192] (8MB), loaded per AllGather chunk.
                lt8 = [
                    ltr.tile([P, 2, N], F8, tag=f"lt{h2}", name=f"lt{h2}")
                    for h2 in range(HT2)
                ]
                for ch in range(NCH):
                    for ht in range(HT):
                        dst = lt8[ht // 2][:, ht % 2, :].rearrange(
                            "p (c g n) -> p c g n", c=NCORES, g=NCH
                        )[:, :, ch, :]
                        nc.sync.dma_start(
                            out=dst,
                            in_=ag_out[ch][:, ht].rearrange("c p n -> p c n"),
                        )

                # key-tile blocks ordered chunk-half first, so the first half
                # only depends on AllGather 0
                blocks = [
                    [c8 * RTOT + half * RT + jj for c8 in (2 * bc, 2 * bc + 1)
                     for jj in range(RT)]
                    for half in range(NCH)
                    for bc in range(NCORES // 2)
                ]

                # global column-sum row of 32*L: vrow32[0, h] = sum_k 32*L[k, h]
                vs_all = fin.tile([P, HT], F32, tag="vsa")
                nc.sync.dma_start(out=vs_all[:], in_=ar_out[:])
                vrow32 = fin.tile([1, H], F32, tag="vrow")
                for ht in range(HT):
                    vtp = tvs.tile([1, P], F32, tag="tv")
                    nc.tensor.transpose(
                        vtp[:], vs_all[:, ht : ht + 1], ident32[:]
                    )
                    nc.vector.tensor_copy(
                        out=vrow32[0:1, ht * P : (ht + 1) * P], in_=vtp[:]
                    )

                csum = fin.tile([P, NL], F32, tag="csum")
                out_sb = [
                    osb.tile([P, H], F32, tag="o", name="o") for _ in range(QTP)
                ]
                for blk, kts in enumerate(blocks):
                    e8s = []
                    v8s = []
                    for j, kt in enumerate(kts):
                        ksl = slice(kt * P, (kt + 1) * P)
                        if j % 2 == 0:
                            e8 = e8p.tile([P, 2, NL], F8, tag="e8")
                            v8 = vp.tile([P, 2, H], F8, tag="v8")
                            e8s.append(e8)
                            v8s.append(v8)
                            # direct load of the AllGathered V-natural pair
                            c, rem = kt // RTOT, kt % RTOT
                            g, jj = rem // RT, rem % RT
                            nc.gpsimd.dma_start(
                                out=v8[:],
                                in_=agv_out[g][c, jj : jj + 2].rearrange(
                                    "v p n -> p v n"
                                ),
                            )
                        eb = epool.tile([P, NL], BF, tag="e")
                        for qh in range(QH):
                            qsl = slice(qh * CHUNK, (qh + 1) * CHUNK)
                            ps = sps.tile([P, CHUNK], F32, tag="sp")
                            for h2 in range(HT2):
                                nc.tensor.matmul(
                                    ps[:],
                                    lhsT=lt8[h2][:, :, ksl],
                                    rhs=q8[h2][:, :, qsl],
                                    start=(h2 == 0),
                                    stop=(h2 == HT2 - 1),
                                    perf_mode=DR,
                                )
                            nc.scalar.activation(
                                out=eb[:, qsl], in_=ps[:], func=EXP, scale=SCALE8
                            )
                        # denominator partial (cross-partition sum at end)
                        if blk == 0 and kts[0] == kt:
                            nc.vector.tensor_copy(out=csum[:], in_=eb[:])
                        else:
                            nc.vector.tensor_add(out=csum[:], in0=csum[:], in1=eb[:])
                        # e' = KE*(exp(s)-1) in fp8 keeps the softmax signal
                        nc.vector.tensor_scalar(
                            out=e8[:, j % 2, :],
                            in0=eb[:],
                            scalar1=-1.0,
                            scalar2=KE,
                            op0=mybir.AluOpType.add,
                            op1=mybir.AluOpType.mult,
                        )

                    last = blk == NBLK - 1
                    for qt in range(QTP):
                        po = ops.tile([P, H], F32, tag="op")
                        for j2 in range(BLK // 2):
                            for hh in range(HH):
                                nc.tensor.matmul(
                                    po[:, hh * CHUNK : (hh + 1) * CHUNK],
                                    lhsT=e8s[j2][:, :, qt * P : (qt + 1) * P],
                                    rhs=v8s[j2][:, :, hh * CHUNK : (hh + 1) * CHUNK],
                                    start=(j2 == 0),
                                    stop=(j2 == BLK // 2 - 1 and not last),
                                    perf_mode=DR,
                                )
                        if last:
                            # exact rank-1 term: out_unnorm*32*KE needs
                            # + KE * sum_k 32*L[k, :] added once per chain
                            for hh in range(HH):
                                nc.tensor.matmul(
                                    po[:, hh * CHUNK : (hh + 1) * CHUNK],
                                    lhsT=kerow32[0:1, :],
                                    rhs=vrow32[0:1, hh * CHUNK : (hh + 1) * CHUNK],
                                    start=False,
                                    stop=True,
                                )
                        if blk == 0:
                            nc.vector.tensor_copy(out=out_sb[qt][:], in_=po[:])
                        else:
                            nc.vector.tensor_add(
                                out=out_sb[qt][:], in0=out_sb[qt][:], in1=po[:]
                            )
                # normalize: colsum -> reciprocal row, move q to partitions.
                # V carried x32 and e' x KE (one1 = 1/(32*KE) folds both back).
                rec_row = fin.tile([1, NL], F32, tag="rr")
                for qh in range(QH):
                    qsl = slice(qh * CHUNK, (qh + 1) * CHUNK)
                    psc = sps.tile([1, CHUNK], F32, tag="sp")
                    nc.tensor.matmul(
                        psc[:], lhsT=ones32[:], rhs=csum[:, qsl],
                        start=True, stop=True,
                    )
                    nc.vector.reciprocal(rec_row[0:1, qsl], psc[:])
                for qt in range(QTP):
                    ct = tvs.tile([P, 1], F32, tag="tv")
                    nc.tensor.matmul(
                        ct[:],
                        lhsT=rec_row[0:1, qt * P : (qt + 1) * P],
                        rhs=one1[0:1, 0:1],
                        start=True,
                        stop=True,
                    )
                    rec = fin.tile([P, 1], F32, tag="rec")
                    nc.vector.tensor_copy(out=rec[:], in_=ct[:])
                    nc.vector.tensor_mul(
                        out=out_sb[qt][:],
                        in0=out_sb[qt][:],
                        in1=rec[:, 0:1].to_broadcast([P, H]),
                    )
                    nc.sync.dma_start(
                        out=out[qt * P : (qt + 1) * P, :], in_=out_sb[qt][:]
                    )
    nc.finalize()
    return nc


def _prep_inputs(inputs):
    import ml_dtypes

    f8 = ml_dtypes.float8_e4m3
    ids = np.asarray(inputs["input_ids"]).astype(np.int32)
    pids = np.asarray(inputs["pos_ids"]).astype(np.int32)
    emb = (np.asarray(inputs["emb"], dtype=np.float32) * FSCALE).astype(f8)
    pemb = (np.asarray(inputs["pos_emb"], dtype=np.float32) * FSCALE).astype(f8)
    W = np.asarray(inputs["W"], dtype=np.float32)
    b = np.asarray(inputs["b"], dtype=np.float32)
    wt = np.ascontiguousarray((W.T * FSCALE).astype(f8))  # [2H, H]
    bias = np.ascontiguousarray(b.reshape(HT, P, 1))
    in_maps = []
    for i in range(NCORES):
        sl = slice(i * NL, (i + 1) * NL)
        in_maps.append(
            {
                "ids": np.ascontiguousarray(ids[sl].reshape(RTOT, P, 1)),
                "pids": np.ascontiguousarray(pids[sl].reshape(RTOT, P, 1)),
                "emb": emb,
                "pemb": pemb,
                "wt": wt,
                "bias": bias,
            }
        )
    return in_maps


def run(inputs, trace=False):
    nc = build_nc()
    in_maps = _prep_inputs(inputs)
    res = run_bass_kernel_spmd(nc, in_maps, list(range(NCORES)), trace=trace)
    out = np.concatenate([res.results[i]["out"] for i in range(NCORES)], axis=0)
    return out, res


def kernel(**inputs):
    out, _ = run(inputs, trace=False)
    return out


# revision 38
# speedup vs baseline: 2.7350x; 1.0997x over previous
"""Trainium2 Bass kernel for nn_AttentiveEncoderPOS (embed+concat+linear+self-attention).

Strategy (8 cores, sequence-parallel with AllGather):
  - Each core gathers/computes only ITS 1024-row slice of
    L = concat(emb[ids], pos[pids]) @ W.T + b, in transposed layout
    (L.T, h on partitions), quantized to fp8 (x32 scale). A per-chunk
    8-core AllGather (fp8 payload) shares all slices while compute runs.
  - Full fp8 L.T stays resident in SBUF in DoubleRow layout [128, 2, N]
    (adjacent h-tile pairs stacked), so score matmuls run fp8 DoubleRow
    (2 contraction rows/cycle). V tiles are transposed out of the same
    resident fp8 L.T on the PE and widened to bf16; exp() output is bf16,
    so attn @ V runs bf16. Phase 2 streams nothing from DRAM.
  - Scores are tiny (|s|<0.025) so exp() without max-subtraction is exact
    softmax; denominator accumulates on the vector engine.
"""

import numpy as np

import concourse.bass as bass
import concourse.mybir as mybir
from concourse import bacc
from concourse.tile import TileContext
from concourse.bass_utils import run_bass_kernel_spmd
from concourse.masks import make_identity

N = 8192
H = 1024
VOCAB = 50257
POS = 64
NCORES = 8
NL = N // NCORES          # 1024 rows (queries) per core
P = 128
HT = H // P               # 8 h tiles
HT2 = HT // 2             # 4 DoubleRow h-pair tiles
K2 = 2 * H
KTI = K2 // P             # 16 contraction tiles for the linear
RTOT = NL // P            # 8 row tiles per core
CHUNK = 512
NCH = NL // CHUNK         # 2 phase-1 chunks
RT = CHUNK // P           # 4 row tiles / chunk
KT = N // P               # 64 key tiles
BLK = 8                   # key tiles per phase-2 block
NBLK = KT // BLK
QTP = NL // P             # 8 q tiles
QH = NL // CHUNK          # 2 score chunks along q
HH = H // CHUNK           # 2 A@V output chunks
FSCALE = 32.0             # fp8 quantization scale for L
KE = 8.0                  # fp8 scale for e' = KE*(exp(s)-1)
SCALE = 1.0 / 32.0        # 1/sqrt(H)
SCALE8 = SCALE / (FSCALE * FSCALE)

BF = mybir.dt.bfloat16
F8 = mybir.dt.float8e4
F32 = mybir.dt.float32
I32 = mybir.dt.int32
EXP = mybir.ActivationFunctionType.Exp
COPY = mybir.ActivationFunctionType.Copy
DR = mybir.MatmulPerfMode.DoubleRow


def build_nc():
    nc = bacc.Bacc()
    ids = nc.declare_dram_parameter("ids", [RTOT, P, 1], I32, isOutput=False)
    pids = nc.declare_dram_parameter("pids", [RTOT, P, 1], I32, isOutput=False)
    # emb/pemb/wt arrive pre-quantized to fp8 (x32) from the host
    emb = nc.declare_dram_parameter("emb", [VOCAB, H], F8, isOutput=False)
    pemb = nc.declare_dram_parameter("pemb", [POS, H], F8, isOutput=False)
    wt = nc.declare_dram_parameter("wt", [K2, H], F8, isOutput=False)  # 32*W.T
    bias = nc.declare_dram_parameter("bias", [HT, P, 1], F32, isOutput=False)
    out = nc.declare_dram_parameter("out", [NL, H], F32, isOutput=True)

    # AllGather bounce buffers (fp8), one pair per 512-row phase-1 chunk so
    # the gather of chunk 0 overlaps phase-1 compute of chunk 1 and phase 2.
    # L.T tiles and V-natural row tiles travel in separate collectives so
    # score matmuls only wait on the L gather.
    ag_in = [nc.dram_tensor(f"ag_in{c}", [HT, P, CHUNK], F8) for c in range(NCH)]
    ag_out = [
        nc.dram_tensor(
            f"ag_out{c}", [NCORES, HT, P, CHUNK], F8, addr_space="Shared"
        )
        for c in range(NCH)
    ]
    agv_in = [nc.dram_tensor(f"agv_in{c}", [RT, P, H], F8) for c in range(NCH)]
    agv_out = [
        nc.dram_tensor(
            f"agv_out{c}", [NCORES, RT, P, H], F8, addr_space="Shared"
        )
        for c in range(NCH)
    ]
    # AllReduce for the global column-sum of 32*L (the attn@V fp8 correction)
    ar_in = nc.dram_tensor("ar_in", [P, HT], F32)
    ar_out = nc.dram_tensor("ar_out", [P, HT], F32, addr_space="Shared")

    with TileContext(nc) as tc:
        with (
            tc.tile_pool(name="const", bufs=1) as const,
            tc.tile_pool(name="qres", bufs=1) as qresp,
        ):
            ident8 = const.tile([P, P], F8)
            make_identity(nc, ident8[:])
            ident32 = const.tile([P, P], F32)
            make_identity(nc, ident32[:])
            ones32 = const.tile([P, 1], F32)
            nc.gpsimd.memset(ones32[:], 1.0)
            kerow32 = const.tile([1, P], F32)
            nc.gpsimd.memset(kerow32[:], KE)
            one1 = const.tile([1, 1], F32)
            nc.gpsimd.memset(one1[:], 1.0 / (FSCALE * KE))
            b_sb = const.tile([P, HT], F32)
            nc.sync.dma_start(
                out=b_sb[:].rearrange("p (h u) -> p h u", h=HT),
                in_=bias.rearrange("h p u -> p h u"),
            )
            # the fp8 linear computes 1024*(X@W.T); fold bias pre-scaled
            b1024_sb = const.tile([P, HT], F32)
            nc.vector.tensor_scalar_mul(
                out=b1024_sb[:], in0=b_sb[:], scalar1=FSCALE * FSCALE
            )
            # own fp8 L.T chunk in DoubleRow layout (these are the queries)
            q8 = [
                qresp.tile([P, 2, NL], F8, tag=f"q{h2}", name=f"q{h2}")
                for h2 in range(HT2)
            ]

            # ---------------- Phase 1: own L.T chunk ----------------
            with (
                tc.tile_pool(name="wtp", bufs=1) as wtp,
                tc.tile_pool(name="idp", bufs=8) as idp,
                tc.tile_pool(name="xfp", bufs=RTOT + 1) as xfp,
                tc.tile_pool(name="xbp", bufs=RT + 1) as xbp,
                tc.tile_pool(name="xtp", bufs=KTI + 2) as xtp,
                tc.tile_pool(name="tps", bufs=2, space="PSUM") as tps,
                tc.tile_pool(name="mps", bufs=2, space="PSUM") as mps,
            ):
                # ids first, then chunk-0 gathers, so the W loads (on the
                # scalar queue) don't gate the first AllGather.
                idts, pidts = [], []
                for t in range(RTOT):
                    idt = idp.tile([P, 1], I32, tag="id")
                    nc.sync.dma_start(out=idt[:], in_=ids[t])
                    pidt = idp.tile([P, 1], I32, tag="pid")
                    nc.sync.dma_start(out=pidt[:], in_=pids[t])
                    idts.append(idt)
                    pidts.append(pidt)

                # gathers for all row tiles (fp8 tables) issue first, then W
                xfs = []
                for t in range(RTOT):
                    xf = xfp.tile([P, K2], F8, tag="xf")
                    nc.gpsimd.indirect_dma_start(
                        out=xf[:, 0:H],
                        out_offset=None,
                        in_=emb[:],
                        in_offset=bass.IndirectOffsetOnAxis(
                            ap=idts[t][:, :1], axis=0
                        ),
                    )
                    nc.gpsimd.indirect_dma_start(
                        out=xf[:, H:K2],
                        out_offset=None,
                        in_=pemb[:],
                        in_offset=bass.IndirectOffsetOnAxis(
                            ap=pidts[t][:, :1], axis=0
                        ),
                    )
                    xfs.append(xf)

                # 32*W.T, fp8, straight into DoubleRow layout
                w8 = []
                for k in range(KTI):
                    if k % 2 == 0:
                        w8.append(
                            wtp.tile([P, 2, H], F8, tag=f"w8_{k//2}", name=f"w8_{k//2}")
                        )
                    nc.scalar.dma_start(
                        out=w8[k // 2][:, k % 2, :], in_=wt[k * P : (k + 1) * P, :]
                    )

                for ch in range(NCH):
                    x8bs = [xfs[ch * RT + rt] for rt in range(RT)]
                    x8ts = []
                    for k2 in range(KTI // 2):
                        x8t = xtp.tile([P, 2, CHUNK], F8, tag="xt")
                        for r in range(2):
                            pt = tps.tile([P, CHUNK, 2], F8, tag="tp")
                            k = 2 * k2 + r
                            for rt in range(RT):
                                nc.tensor.transpose(
                                    pt[:, rt * P : (rt + 1) * P, 0],
                                    x8bs[rt][:, k * P : (k + 1) * P],
                                    ident8[:],
                                )
                            nc.scalar.activation(
                                out=x8t[:, r, :], in_=pt[:, :, 0], func=COPY
                            )
                        x8ts.append(x8t)

                    # linear (fp8 DR): 1024*L.T[ht, chunk]
                    csl = slice(ch * CHUNK, (ch + 1) * CHUNK)
                    for ht in range(HT):
                        pm = mps.tile([P, CHUNK], F32, tag="mp")
                        for k2 in range(KTI // 2):
                            nc.tensor.matmul(
                                pm[:],
                                lhsT=w8[k2][:, :, ht * P : (ht + 1) * P],
                                rhs=x8ts[k2][:],
                                start=(k2 == 0),
                                stop=(k2 == KTI // 2 - 1),
                                perf_mode=DR,
                            )
                        # fp8 quantize: q8 = 32*(pm/1024 + b) = (pm + 1024b)/32
                        nc.vector.tensor_scalar(
                            out=q8[ht // 2][:, ht % 2, csl],
                            in0=pm[:],
                            scalar1=b1024_sb[:, ht : ht + 1],
                            scalar2=1.0 / FSCALE,
                            op0=mybir.AluOpType.add,
                            op1=mybir.AluOpType.mult,
                        )
                        nc.sync.dma_start(
                            out=ag_in[ch][ht], in_=q8[ht // 2][:, ht % 2, csl]
                        )
                    # AllGather the L.T chunk first: scores only need this one
                    nc.gpsimd.collective_compute(
                        "AllGather",
                        mybir.AluOpType.bypass,
                        replica_groups=[list(range(NCORES))],
                        ins=[ag_in[ch][:].opt()],
                        outs=[ag_out[ch][:].opt()],
                    )
                    # V-natural tiles for this chunk (so phase 2 needn't
                    # transpose): transpose own L.T rows back to [keys, h]
                    for rt in range(RT):
                        rsl = slice(ch * CHUNK + rt * P, ch * CHUNK + (rt + 1) * P)
                        ptv = tps.tile([P, H, 2], F8, tag="tpv")
                        for ht in range(HT):
                            nc.tensor.transpose(
                                ptv[:, ht * P : (ht + 1) * P, 0],
                                q8[ht // 2][:, ht % 2, rsl],
                                ident8[:],
                            )
                        vn = xbp.tile([P, H], F8, tag="vn", bufs=3)
                        nc.scalar.activation(
                            out=vn[:], in_=ptv[:, :, 0], func=COPY
                        )
                        nc.sync.dma_start(out=agv_in[ch][rt], in_=vn[:])
                    nc.gpsimd.collective_compute(
                        "AllGather",
                        mybir.AluOpType.bypass,
                        replica_groups=[list(range(NCORES))],
                        ins=[agv_in[ch][:].opt()],
                        outs=[agv_out[ch][:].opt()],
                    )

                # own-chunk column sums of 32*L (for the attn@V correction):
                # sum q8 over keys on the scalar engine, AllReduce across cores
                vs_own = xtp.tile([P, HT], F32, tag="vso", bufs=1)
                for ht in range(HT):
                    scr8 = xbp.tile([P, NL], F8, tag="scr8", bufs=2)
                    nc.scalar.activation(
                        out=scr8[:],
                        in_=q8[ht // 2][:, ht % 2, :],
                        func=COPY,
                        accum_out=vs_own[:, ht : ht + 1],
                    )
                nc.sync.dma_start(out=ar_in[:], in_=vs_own[:])
                nc.gpsimd.collective_compute(
                    "AllReduce",
                    mybir.AluOpType.add,
                    replica_groups=[list(range(NCORES))],
                    ins=[ar_in[:].opt()],
                    outs=[ar_out[:].opt()],
                )

            # ---------------- Phase 2: attention ----------------
            with (
                tc.tile_pool(name="ltr", bufs=1) as ltr,
                tc.tile_pool(name="ep", bufs=6) as epool,
                tc.tile_pool(name="e8p", bufs=BLK + 2) as e8p,
                tc.tile_pool(name="vp", bufs=BLK + 2) as vp,
                tc.tile_pool(name="osb", bufs=QTP) as osb,
                tc.tile_pool(name="fin", bufs=2) as fin,
                tc.tile_pool(name="sps", bufs=3, space="PSUM") as sps,
                tc.tile_pool(name="ops", bufs=2, space="PSUM") as ops,
                tc.tile_pool(name="tvs", bufs=1, space="PSUM") as tvs,
            ):
                # Full fp8 L.T resident in DoubleRow layout: 4 tiles
                # [128, 2, 8192] (8MB), loaded per AllGather chunk.
                lt8 = [
                    ltr.tile([P, 2, N], F8, tag=f"lt{h2}", name=f"lt{h2}")
                    for h2 in range(HT2)
                ]
                for ch in range(NCH):
                    for ht in range(HT):
                        dst = lt8[ht // 2][:, ht % 2, :].rearrange(
                            "p (c g n) -> p c g n", c=NCORES, g=NCH
                        )[:, :, ch, :]
                        nc.sync.dma_start(
                            out=dst,
                            in_=ag_out[ch][:, ht].rearrange("c p n -> p c n"),
                        )

                # key-tile blocks ordered chunk-half first, so the first half
                # only depends on AllGather 0
                blocks = [
                    [c8 * RTOT + half * RT + jj for c8 in (2 * bc, 2 * bc + 1)
                     for jj in range(RT)]
                    for half in range(NCH)
                    for bc in range(NCORES // 2)
                ]

                # global column-sum row of 32*L: vrow32[0, h] = sum_k 32*L[k, h]
                vs_all = fin.tile([P, HT], F32, tag="vsa")
                nc.sync.dma_start(out=vs_all[:], in_=ar_out[:])
                vrow32 = fin.tile([1, H], F32, tag="vrow")
                for ht in range(HT):
                    vtp = tvs.tile([1, P], F32, tag="tv")
                    nc.tensor.transpose(
                        vtp[:], vs_all[:, ht : ht + 1], ident32[:]
                    )
                    nc.vector.tensor_copy(
                        out=vrow32[0:1, ht * P : (ht + 1) * P], in_=vtp[:]
                    )

                csum = fin.tile([P, NL], F32, tag="csum")
                out_sb = [
                    osb.tile([P, H], F32, tag="o", name="o") for _ in range(QTP)
                ]
                for blk, kts in enumerate(blocks):
                    e8s = []
                    v8s = []
                    for j, kt in enumerate(kts):
                        ksl = slice(kt * P, (kt + 1) * P)
                        if j % 2 == 0:
                            e8 = e8p.tile([P, 2, NL], F8, tag="e8")
                            v8 = vp.tile([P, 2, H], F8, tag="v8")
                            e8s.append(e8)
                            v8s.append(v8)
                            # direct load of the AllGathered V-natural pair
                            c, rem = kt // RTOT, kt % RTOT
                            g, jj = rem // RT, rem % RT
                            nc.gpsimd.dma_start(
                                out=v8[:],
                                in_=agv_out[g][c, jj : jj + 2].rearrange(
                                    "v p n -> p v n"
                                ),
                            )
                        eb = epool.tile([P, NL], BF, tag="e")
                        for qh in range(QH):
                            qsl = slice(qh * CHUNK, (qh + 1) * CHUNK)
                            ps = sps.tile([P, CHUNK], F32, tag="sp")
                            for h2 in range(HT2):
                                nc.tensor.matmul(
                                    ps[:],
                                    lhsT=lt8[h2][:, :, ksl],
                                    rhs=q8[h2][:, :, qsl],
                                    start=(h2 == 0),
                                    stop=(h2 == HT2 - 1),
                                    perf_mode=DR,
                                )
                            nc.scalar.activation(
                                out=eb[:, qsl], in_=ps[:], func=EXP, scale=SCALE8
                            )
                        # denominator partial (cross-partition sum at end)
                        if blk == 0 and kts[0] == kt:
                            nc.vector.tensor_copy(out=csum[:], in_=eb[:])
                        else:
                            nc.vector.tensor_add(out=csum[:], in0=csum[:], in1=eb[:])
                        # e' = KE*(exp(s)-1) in fp8 keeps the softmax signal
                        nc.vector.tensor_scalar(
                            out=e8[:, j % 2, :],
                            in0=eb[:],
                            scalar1=-1.0,
                            scalar2=KE,
                            op0=mybir.AluOpType.add,
                            op1=mybir.AluOpType.mult,
                        )

                    # the rank-1 correction joins block NBLK-2's chains: late
                    # enough for the AllReduce, off the critical final block
                    corr = blk == NBLK - 2
                    for qt in range(QTP):
                        po = ops.tile([P, H], F32, tag="op")
                        for j2 in range(BLK // 2):
                            for hh in range(HH):
                                nc.tensor.matmul(
                                    po[:, hh * CHUNK : (hh + 1) * CHUNK],
                                    lhsT=e8s[j2][:, :, qt * P : (qt + 1) * P],
                                    rhs=v8s[j2][:, :, hh * CHUNK : (hh + 1) * CHUNK],
                                    start=(j2 == 0),
                                    stop=(j2 == BLK // 2 - 1 and not corr),
                                    perf_mode=DR,
                                )
                        if corr:
                            # exact rank-1 term: out_unnorm*32*KE needs
                            # + KE * sum_k 32*L[k, :] added once per chain
                            for hh in range(HH):
                                nc.tensor.matmul(
                                    po[:, hh * CHUNK : (hh + 1) * CHUNK],
                                    lhsT=kerow32[0:1, :],
                                    rhs=vrow32[0:1, hh * CHUNK : (hh + 1) * CHUNK],
                                    start=False,
                                    stop=True,
                                )
                        if blk == 0:
                            nc.vector.tensor_copy(out=out_sb[qt][:], in_=po[:])
                        else:
                            nc.vector.tensor_add(
                                out=out_sb[qt][:], in0=out_sb[qt][:], in1=po[:]
                            )
                # normalize: colsum -> reciprocal row, move q to partitions.
                # V carried x32 and e' x KE (one1 = 1/(32*KE) folds both back).
                rec_row = fin.tile([1, NL], F32, tag="rr")
                for qh in range(QH):
                    qsl = slice(qh * CHUNK, (qh + 1) * CHUNK)
                    psc = sps.tile([1, CHUNK], F32, tag="sp")
                    nc.tensor.matmul(
                        psc[:], lhsT=ones32[:], rhs=csum[:, qsl],
                        start=True, stop=True,
                    )
                    nc.vector.reciprocal(rec_row[0:1, qsl], psc[:])
                for qt in range(QTP):
                    ct = tvs.tile([P, 1], F32, tag="tv")
                    nc.tensor.matmul(
                        ct[:],
                        lhsT=rec_row[0:1, qt * P : (qt + 1) * P],
                        rhs=one1[0:1, 0:1],
                        start=True,
                        stop=True,
                    )
                    rec = fin.tile([P, 1], F32, tag="rec")
                    nc.vector.tensor_copy(out=rec[:], in_=ct[:])
                    nc.vector.tensor_mul(
                        out=out_sb[qt][:],
                        in0=out_sb[qt][:],
                        in1=rec[:, 0:1].to_broadcast([P, H]),
                    )
                    nc.sync.dma_start(
                        out=out[qt * P : (qt + 1) * P, :], in_=out_sb[qt][:]
                    )
    nc.finalize()
    return nc


def _prep_inputs(inputs):
    import ml_dtypes

    f8 = ml_dtypes.float8_e4m3
    ids = np.asarray(inputs["input_ids"]).astype(np.int32)
    pids = np.asarray(inputs["pos_ids"]).astype(np.int32)
    emb = (np.asarray(inputs["emb"], dtype=np.float32) * FSCALE).astype(f8)
    pemb = (np.asarray(inputs["pos_emb"], dtype=np.float32) * FSCALE).astype(f8)
    W = np.asarray(inputs["W"], dtype=np.float32)
    b = np.asarray(inputs["b"], dtype=np.float32)
    wt = np.ascontiguousarray((W.T * FSCALE).astype(f8))  # [2H, H]
    bias = np.ascontiguousarray(b.reshape(HT, P, 1))
    in_maps = []
    for i in range(NCORES):
        sl = slice(i * NL, (i + 1) * NL)
        in_maps.append(
            {
                "ids": np.ascontiguousarray(ids[sl].reshape(RTOT, P, 1)),
                "pids": np.ascontiguousarray(pids[sl].reshape(RTOT, P, 1)),
                "emb": emb,
                "pemb": pemb,
                "wt": wt,
                "bias": bias,
            }
        )
    return in_maps


def run(inputs, trace=False):
    nc = build_nc()
    in_maps = _prep_inputs(inputs)
    res = run_bass_kernel_spmd(nc, in_maps, list(range(NCORES)), trace=trace)
    out = np.concatenate([res.results[i]["out"] for i in range(NCORES)], axis=0)
    return out, res


def kernel(**inputs):
    out, _ = run(inputs, trace=False)
    return out
